# revision 29
# baseline (speedup 1.0000x reference)
"""Trainium2 Bass kernel for nn_MeshNodeBlock (GNN message passing block).

reference semantics:
    agg = segment_sum(edge_features, src_indices, N)        # scatter-add
    x   = concat([node_features, agg], -1)
    h   = silu(x @ W1 + b1)
    y   = h @ W2 + b2
    y   = layer_norm(y) * gamma + beta
    out = y + node_features

Strategy (8 NeuronCores, SPMD, one NEFF):
  * Host graph-partitions nodes contiguously across cores (12800 node slots
    per core) and stable-sorts edges by destination node; each core receives
    exactly the edge rows destined for its nodes, grouped by 128-node tile
    and padded to a per-tile-position chunk count C_i (shared across cores
    so the SPMD program is uniform; pad rows are zero).
  * Device works fully in transposed space (features on partitions, nodes on
    free dim). Per 128-node tile the scatter-add is C_i PE matmuls
    aggT += edge_chunk.T @ onehot. One-hot blocks for a whole tile are built
    in one 2x-mode vector is_equal against a tiled-iota constant, with the
    local ids pre-expanded by a gpsimd broadcast copy.
  * MLP consumes aggT/nodeT directly: layer 1 -> hT_j slices, silu(+b1) on
    the scalar engine, layer 2 -> yT.
  * LayerNorm stats via matmuls whose lhsT is a block-diagonal 1/128 column
    (ONCB): group g's mean/mean-of-squares land on PSUM row g of a shared
    bank, accumulated over a block of groups. Stats post-processing
    (var, rstd=exp(-0.5*ln(var+eps))) runs once per block at full width,
    then rows bounce through a DRAM tile and DMA-broadcast back across
    partitions. Processing is phase-blocked to minimize ACT table switches.
  * Output written transposed in bf16; host transposes/casts back.
"""

import functools
from contextlib import ExitStack

import numpy as np
import ml_dtypes

import concourse.bass as bass
import concourse.tile as tile
from concourse import bacc, mybir
from concourse import bass_utils

BF16 = ml_dtypes.bfloat16
FP8 = ml_dtypes.float8_e4m3

N_NODES = 100000
D = 128
N_CORES = 8
P = 128
GROUP = 512              # nodes per group = 4 tiles
NODES_PER_CORE = 12800   # 25 groups
C_MAX = 8                # fallback chunk budget per tile (exact counts used)
NBLK = 2                 # phase blocks
INTERLEAVE_P3 = False    # interleave prev block's normalize into next phase1
EPS = 1e-5

AF = mybir.ActivationFunctionType
ALU = mybir.AluOpType
dt = mybir.dt


# --------------------------------------------------------------------------
# device kernel builder
# --------------------------------------------------------------------------

@functools.lru_cache(maxsize=4)
def _build(nodes_per_core: int, cis: tuple, n_cores: int, act: str = "silu"):
    assert nodes_per_core % GROUP == 0
    n_groups = nodes_per_core // GROUP
    tiles_per_core = nodes_per_core // P
    assert len(cis) == tiles_per_core
    coff = np.concatenate([[0], np.cumsum(cis)]).astype(int)
    ch = int(coff[-1])                   # total chunks per core
    cmaxt = int(max(cis))

    # phase blocks of groups (ACT table switches cost ~2.7us per set swap).
    # Asymmetric: big first block, small last block whose normalize tail is
    # all that remains after PE finishes.
    if n_groups >= 8:
        ntail = max(4, n_groups // 4)
        blocks = [list(range(0, n_groups - ntail)),
                  list(range(n_groups - ntail, n_groups))]
    else:
        blocks = [list(range(n_groups))]
    bmax = max(len(b) for b in blocks)

    nc = bacc.Bacc("TRN2", target_bir_lowering=False, debug=False,
                   enable_asserts=False, num_devices=n_cores)

    EB = nc.dram_tensor("eb", [P, ch * 128], dt.bfloat16, kind="ExternalInput").ap()
    OHD = nc.dram_tensor("ohd", [P, ch * 128], dt.float8e4,
                         kind="ExternalInput").ap()
    NTB = nc.dram_tensor("ntb", [P, nodes_per_core], dt.bfloat16,
                         kind="ExternalInput").ap()
    NPB = nc.dram_tensor("npb", [P, nodes_per_core], dt.bfloat16,
                         kind="ExternalInput").ap()
    W1P = nc.dram_tensor("w1p", [P, 1024], dt.bfloat16, kind="ExternalInput").ap()
    W2P = nc.dram_tensor("w2p", [P, 512], dt.bfloat16, kind="ExternalInput").ap()
    B1P = nc.dram_tensor("b1p", [P, 4], dt.float32, kind="ExternalInput").ap()
    B2P = nc.dram_tensor("b2p", [P, 1], dt.float32, kind="ExternalInput").ap()
    GAM = nc.dram_tensor("gam", [P, 1], dt.float32, kind="ExternalInput").ap()
    BET = nc.dram_tensor("bet", [P, 1], dt.float32, kind="ExternalInput").ap()
    ONB = nc.dram_tensor("onb", [P, bmax * 128], dt.bfloat16,
                         kind="ExternalInput").ap()
    OUT = nc.dram_tensor("out", [P, nodes_per_core], dt.bfloat16,
                         kind="ExternalOutput").ap()

    with tile.TileContext(nc) as tc:
        with ExitStack() as ctx:
            singles = ctx.enter_context(tc.tile_pool(name="singles", bufs=1))
            ebp = ctx.enter_context(tc.tile_pool(name="ebp", bufs=8))
            ohp = ctx.enter_context(tc.tile_pool(name="ohp", bufs=8))
            xtp = ctx.enter_context(tc.tile_pool(name="xtp", bufs=4))
            shp = ctx.enter_context(tc.tile_pool(name="shp", bufs=2))
            yp = ctx.enter_context(tc.tile_pool(name="yp", bufs=n_groups + 2))
            npp = ctx.enter_context(tc.tile_pool(name="npp", bufs=n_groups + 2))
            zp = ctx.enter_context(tc.tile_pool(name="zp", bufs=8))
            stp = ctx.enter_context(tc.tile_pool(name="stp", bufs=2))
            psagg = ctx.enter_context(tc.tile_pool(name="psagg", bufs=2, space="PSUM"))
            psh = ctx.enter_context(tc.tile_pool(name="psh", bufs=3, space="PSUM"))
            psy = ctx.enter_context(tc.tile_pool(name="psy", bufs=1, space="PSUM"))
            psst = ctx.enter_context(tc.tile_pool(name="psst", bufs=1, space="PSUM"))
            drp = ctx.enter_context(tc.tile_pool(name="drp", bufs=2, space="DRAM"))

            def load_const(name, src, shape, dtyp):
                t = singles.tile(shape, dtyp, tag=name)
                nc.sync.dma_start(out=t[:], in_=src)
                return t

            w1 = load_const("w1", W1P, [P, 1024], dt.bfloat16)
            w2 = load_const("w2", W2P, [P, 512], dt.bfloat16)
            b1 = load_const("b1", B1P, [P, 4], dt.float32)
            b2 = load_const("b2", B2P, [P, 1], dt.float32)
            gam = load_const("gam", GAM, [P, 1], dt.float32)
            bet = load_const("bet", BET, [P, 1], dt.float32)
            onb = load_const("onb", ONB, [P, bmax * 128], dt.bfloat16)
            eps = singles.tile([P, 1], dt.float32, tag="eps")
            nc.vector.memset(eps[:], EPS)

            y_tiles = {}
            npb_tiles = {}
            _last_stats = []

            xta_tiles = {}
            xtn_tiles = {}

            def phase1(block, bi, interleave=None):
                bsz = len(block)
                mu_ps = psst.tile([P, GROUP], dt.float32, tag="mups")
                m2_ps = psst.tile([P, GROUP], dt.float32, tag="m2ps")
                for gi, g in enumerate(block):
                    nsl = slice(g * GROUP, (g + 1) * GROUP)
                    xtn = xtp.tile([P, GROUP], dt.bfloat16, tag="xtn")
                    nc.sync.dma_start(out=xtn[:], in_=NTB[:, nsl])
                    xtn_tiles[g] = xtn
                    npbt = npp.tile([P, GROUP], dt.bfloat16, tag="npb")
                    nc.sync.dma_start(out=npbt[:], in_=NPB[:, nsl])
                    npb_tiles[g] = npbt

                    agg_ps = psagg.tile([P, GROUP], dt.float32, tag="agg")
                    for t4 in range(4):
                        ti = g * 4 + t4
                        cw = int(cis[ti]) * 128
                        o0 = int(coff[ti])
                        eb = ebp.tile([P, cmaxt * 128], dt.bfloat16, tag="eb")
                        nc.sync.dma_start(
                            out=eb[:, :cw], in_=EB[:, o0 * 128:o0 * 128 + cw])
                        oh = ohp.tile([P, cmaxt * 128], dt.float8e4, tag="oh")
                        nc.sync.dma_start(
                            out=oh[:, :cw], in_=OHD[:, o0 * 128:o0 * 128 + cw])
                        for c in range(int(cis[ti])):
                            nc.tensor.matmul(
                                out=agg_ps[:, t4 * 128:(t4 + 1) * 128],
                                lhsT=eb[:, c * 128:(c + 1) * 128],
                                rhs=oh[:, c * 128:(c + 1) * 128],
                                start=(c == 0), stop=(c == int(cis[ti]) - 1))
                    xta = xtp.tile([P, GROUP], dt.bfloat16, tag="xta")
                    if g % 2 == 0:
                        nc.scalar.activation(out=xta[:], in_=agg_ps[:], func=AF.Copy)
                    else:
                        nc.vector.tensor_copy(out=xta[:], in_=agg_ps[:])
                    sh_tiles = []
                    for j in range(4):
                        hps = psh.tile([P, GROUP], dt.float32, tag="hps")
                        nc.tensor.matmul(out=hps[:],
                                         lhsT=w1[:, j * 128:(j + 1) * 128],
                                         rhs=xtn[:], start=True, stop=False)
                        nc.tensor.matmul(
                            out=hps[:],
                            lhsT=w1[:, 512 + j * 128:512 + (j + 1) * 128],
                            rhs=xta[:], start=False, stop=True)
                        sh = shp.tile([P, GROUP], dt.bfloat16, tag=f"sh{j}")
                        if act == "silu":
                            nc.scalar.activation(out=sh[:], in_=hps[:],
                                                 func=AF.Silu,
                                                 bias=b1[:, j:j + 1], scale=1.0)
                        else:
                            sg = shp.tile([P, GROUP], dt.float32, tag=f"sg{j}")
                            nc.scalar.activation(out=sg[:], in_=hps[:],
                                                 func=AF.Sigmoid,
                                                 bias=b1[:, j:j + 1], scale=1.0)
                            u = shp.tile([P, GROUP], dt.float32, tag=f"u{j}")
                            nc.vector.tensor_scalar(
                                out=u[:], in0=hps[:], scalar1=b1[:, j:j + 1],
                                scalar2=None, op0=ALU.add)
                            nc.vector.tensor_tensor(out=sh[:], in0=u[:],
                                                    in1=sg[:], op=ALU.mult)
                        sh_tiles.append(sh)

                    yps = psy.tile([P, GROUP], dt.float32, tag="yps")
                    for j in range(4):
                        nc.tensor.matmul(out=yps[:],
                                         lhsT=w2[:, j * 128:(j + 1) * 128],
                                         rhs=sh_tiles[j][:],
                                         start=(j == 0), stop=(j == 3))
                    y = yp.tile([P, GROUP], dt.bfloat16, tag="y")
                    nc.scalar.activation(out=y[:], in_=yps[:], func=AF.Identity,
                                         bias=b2[:, 0:1], scale=1.0)
                    y_tiles[g] = y
                    y2 = zp.tile([P, GROUP], dt.bfloat16, tag="y2")
                    nc.scalar.square(out=y2[:], in_=y[:])
                    onc_g = onb[:, gi * 128:(gi + 1) * 128]
                    nc.tensor.matmul(out=mu_ps[:], lhsT=onc_g, rhs=y[:],
                                     start=(gi == 0), stop=(gi == bsz - 1),
                                     skip_group_check=True)
                    nc.tensor.matmul(out=m2_ps[:], lhsT=onc_g, rhs=y2[:],
                                     start=(gi == 0), stop=(gi == bsz - 1),
                                     skip_group_check=True)
                    if INTERLEAVE_P3 and interleave and gi < len(interleave[0]):
                        phase3_group(interleave[0][gi], gi, interleave[1])
                if interleave:
                    for gi in range(len(block) if INTERLEAVE_P3 else 0,
                                    len(interleave[0])):
                        phase3_group(interleave[0][gi], gi, interleave[1])
                _last_stats.append((mu_ps, m2_ps))

            def phase2(block, bi, mu_ps, m2_ps):
                mu_bf = stp.tile([P, GROUP], dt.bfloat16, tag="mubf")
                nc.scalar.activation(out=mu_bf[:], in_=mu_ps[:], func=AF.Copy)
                m2_bf = stp.tile([P, GROUP], dt.bfloat16, tag="m2bf")
                nc.scalar.activation(out=m2_bf[:], in_=m2_ps[:], func=AF.Copy)
                musq = stp.tile([P, GROUP], dt.bfloat16, tag="musq")
                nc.scalar.square(out=musq[:], in_=mu_bf[:])
                var = stp.tile([P, GROUP], dt.bfloat16, tag="var")
                nc.vector.tensor_tensor(out=var[:], in0=m2_bf[:], in1=musq[:],
                                        op=ALU.subtract)
                lnv = stp.tile([P, GROUP], dt.bfloat16, tag="lnv")
                nc.scalar.activation(out=lnv[:], in_=var[:], func=AF.Ln,
                                     bias=eps[:, 0:1], scale=1.0)
                rstd = stp.tile([P, GROUP], dt.bfloat16, tag="rstd")
                nc.scalar.activation(out=rstd[:], in_=lnv[:], func=AF.Exp,
                                     bias=0.0, scale=-0.5)
                bounce = drp.tile([len(block), 1024], dt.bfloat16, tag="bounce")
                nc.sync.dma_start(out=bounce[:, 0:512],
                                  in_=mu_bf[0:len(block), :])
                nc.sync.dma_start(out=bounce[:, 512:1024],
                                  in_=rstd[0:len(block), :])
                return bounce

            def phase3_group(g, gi, bounce):
                    nsl = slice(g * GROUP, (g + 1) * GROUP)
                    mubc = zp.tile([P, GROUP], dt.bfloat16, tag="mubc")
                    bsl = bounce[gi:gi + 1, 0:512]
                    nc.sync.dma_start(out=mubc[:], in_=bass.AP(
                        tensor=bsl.tensor, offset=bsl.offset,
                        ap=[[0, P], bsl.ap[1]]))
                    rbc = zp.tile([P, GROUP], dt.bfloat16, tag="rbc")
                    bsl2 = bounce[gi:gi + 1, 512:1024]
                    nc.sync.dma_start(out=rbc[:], in_=bass.AP(
                        tensor=bsl2.tensor, offset=bsl2.offset,
                        ap=[[0, P], bsl2.ap[1]]))
                    y = y_tiles.pop(g)
                    npbt = npb_tiles.pop(g)
                    za = zp.tile([P, GROUP], dt.bfloat16, tag="za")
                    nc.vector.tensor_tensor(out=za[:], in0=y[:], in1=mubc[:],
                                            op=ALU.subtract)
                    zb = zp.tile([P, GROUP], dt.bfloat16, tag="zb")
                    nc.vector.tensor_tensor(out=zb[:], in0=za[:], in1=rbc[:],
                                            op=ALU.mult)
                    zc = zp.tile([P, GROUP], dt.bfloat16, tag="zc")
                    nc.vector.tensor_scalar(out=zc[:], in0=zb[:],
                                            scalar1=gam[:, 0:1],
                                            scalar2=bet[:, 0:1],
                                            op0=ALU.mult, op1=ALU.add)
                    of = zp.tile([P, GROUP], dt.bfloat16, tag="of")
                    nc.vector.tensor_tensor(out=of[:], in0=zc[:], in1=npbt[:],
                                            op=ALU.add)
                    nc.sync.dma_start(out=OUT[:, nsl], in_=of[:])

            # emission: P1(b) P2(b) P3(b); P3 is DVE+DMA-only and P1 is
            # PE/ACT/DMA-only, so P3(b) streams on DVE during P1(b+1).
            for bi, block in enumerate(blocks):
                phase1(block, bi)
                mu_ps, m2_ps = _last_stats.pop()
                bounce = phase2(block, bi, mu_ps, m2_ps)
                for gi, g in enumerate(block):
                    phase3_group(g, gi, bounce)

    nc.compile()
    return nc


# --------------------------------------------------------------------------
# host-side sharding / packing
# --------------------------------------------------------------------------

def _preprocess(inputs, n_cores, nodes_per_core):
    nf = np.ascontiguousarray(np.asarray(inputs["node_features"], np.float32))
    ef = np.ascontiguousarray(np.asarray(inputs["edge_features"], np.float32))
    src = np.asarray(inputs["src_indices"]).astype(np.int64)
    W1 = np.asarray(inputs["W1"], np.float32)
    b1 = np.asarray(inputs["b1"], np.float32)
    W2 = np.asarray(inputs["W2"], np.float32)
    b2 = np.asarray(inputs["b2"], np.float32)
    gam = np.asarray(inputs["ln_gamma"], np.float32)
    bet = np.asarray(inputs["ln_beta"], np.float32)

    n_nodes, d = nf.shape
    n_edges = ef.shape[0]
    tiles_per_core = nodes_per_core // P
    n_groups = nodes_per_core // GROUP
    if n_groups >= 8:
        bmax = n_groups - max(4, n_groups // 4)
    else:
        bmax = n_groups

    order = np.argsort(src, kind="stable")
    snode = src[order]
    core = snode // nodes_per_core
    tile_in_core = (snode % nodes_per_core) // P
    lid = snode % P
    pt = core * tiles_per_core + tile_in_core
    counts = np.bincount(pt, minlength=n_cores * tiles_per_core)
    # per-position chunk counts, shared across cores (SPMD uniformity)
    ccounts = np.ceil(counts.reshape(n_cores, tiles_per_core) / P).astype(int)
    cis = np.maximum(ccounts.max(axis=0), 1)
    coff = np.concatenate([[0], np.cumsum(cis)]).astype(int)
    ch = int(coff[-1])
    cmaxt = int(cis.max())

    starts = np.zeros(n_cores * tiles_per_core, np.int64)
    np.cumsum(counts[:-1], out=starts[1:])
    rank = np.arange(n_edges, dtype=np.int64) - starts[pt]
    chunk = rank // P
    p = rank % P
    cg = coff[tile_in_core] + chunk
    row = core * (P * ch) + p * ch + cg

    ebuf = np.zeros((n_cores * P * ch, d), np.float32)
    ebuf[row] = ef[order]
    EBa = ebuf.reshape(n_cores, P, ch * d).astype(BF16)
    ohbuf = np.zeros((n_cores * P * ch, 128), FP8)
    ohbuf[row, lid] = 1.0
    OHa = ohbuf.reshape(n_cores, P, ch * 128)

    nfp = np.zeros((n_cores * nodes_per_core, d), np.float32)
    nfp[:n_nodes] = nf
    NTBa = np.ascontiguousarray(
        nfp.reshape(n_cores, nodes_per_core, d).transpose(0, 2, 1)).astype(BF16)
    nfp[:n_nodes] = nf + bet[None, :]
    nfp[n_nodes:] = bet[None, :]
    NPBa = np.ascontiguousarray(
        nfp.reshape(n_cores, nodes_per_core, d).transpose(0, 2, 1)).astype(BF16)

    W1P = np.ascontiguousarray(
        W1.reshape(2, P, 4, P).transpose(1, 0, 2, 3).reshape(P, 1024)).astype(BF16)
    W2P = np.ascontiguousarray(
        W2.reshape(4, P, P).transpose(1, 0, 2).reshape(P, 512)).astype(BF16)
    B1P = np.ascontiguousarray(b1.reshape(4, P).T)
    B2P = np.ascontiguousarray(b2.reshape(P, 1))
    GAMP = np.ascontiguousarray(gam.reshape(P, 1))
    # beta is folded into NPB; device beta input stays zero
    BETP = np.zeros((P, 1), np.float32)
    ONB = np.zeros((P, bmax * 128), np.float32)
    for g in range(bmax):
        ONB[:, g * 128 + g] = 1.0 / P
    ONB = ONB.astype(BF16)

    in_maps = []
    for k in range(n_cores):
        in_maps.append({
            "eb": EBa[k], "ohd": OHa[k], "ntb": NTBa[k], "npb": NPBa[k],
            "w1p": W1P, "w2p": W2P, "b1p": B1P, "b2p": B2P,
            "gam": GAMP, "bet": BETP, "onb": ONB,
        })
    return in_maps, tuple(int(c) for c in cis)


def _assemble(results, n_nodes, n_cores, nodes_per_core):
    outs = np.stack([np.asarray(r["out"]) for r in results])
    full = outs.astype(np.float32).transpose(0, 2, 1).reshape(
        n_cores * nodes_per_core, -1)
    return np.ascontiguousarray(full[:n_nodes])


# --------------------------------------------------------------------------
# public entry point
# --------------------------------------------------------------------------

ACT_MODE = "silu"

_AXON_SO = "/opt/axon/libaxon_pjrt.so"


def _ensure_ntff_hook():
    """Provide antenv.axon_hooks + register the ctypes NTFF profile hook
    (the agent image's antenv lacks axon_hooks, so boot degraded silently)."""
    import sys
    import types
    import ctypes
    import contextlib
    import os

    try:
        from antenv.axon_hooks import get_axon_ntff_profile_hook  # noqa: F401
        return
    except ImportError:
        pass
    import antenv

    m = types.ModuleType("antenv.axon_hooks")
    m._hook = None

    def set_axon_ntff_profile_hook(h):
        m._hook = h

    def get_axon_ntff_profile_hook():
        return m._hook

    m.set_axon_ntff_profile_hook = set_axon_ntff_profile_hook
    m.get_axon_ntff_profile_hook = get_axon_ntff_profile_hook
    sys.modules["antenv.axon_hooks"] = m
    antenv.axon_hooks = m

    if not os.path.exists(_AXON_SO):
        return
    lib = ctypes.CDLL(_AXON_SO)
    if not hasattr(lib, "axon_start_nrt_profile"):
        return
    lib.axon_start_nrt_profile.argtypes = [ctypes.POINTER(ctypes.c_int64),
                                           ctypes.c_size_t]
    lib.axon_start_nrt_profile.restype = ctypes.c_int64
    lib.axon_stop_nrt_profile.argtypes = [ctypes.c_char_p]
    lib.axon_stop_nrt_profile.restype = ctypes.c_int64

    @contextlib.contextmanager
    def _hook(output_dir, device_ids):
        import jax

        jax.devices()
        if device_ids:
            ids = (ctypes.c_int64 * len(device_ids))(*device_ids)
            rc = lib.axon_start_nrt_profile(ids, len(device_ids))
        else:
            rc = lib.axon_start_nrt_profile(None, 0)
        if rc != 0:
            raise RuntimeError(f"axon_start_nrt_profile rc={rc}")
        try:
            yield
        finally:
            n = lib.axon_stop_nrt_profile(str(output_dir).encode())
            if n < 0:
                raise RuntimeError(f"axon_stop_nrt_profile rc={n}")
            if n == 0:
                print("WARNING: NTFF capture wrote no files")

    m._hook = _hook


def _run(inputs, trace=False):
    if trace:
        _ensure_ntff_hook()
    n_nodes = np.asarray(inputs["node_features"]).shape[0]
    in_maps, cis = _preprocess(inputs, N_CORES, NODES_PER_CORE)
    nc = _build(NODES_PER_CORE, cis, N_CORES, ACT_MODE)
    res = bass_utils.run_bass_kernel_spmd(
        nc, in_maps, core_ids=list(range(N_CORES)), trace=trace)
    out = _assemble(res.results, n_nodes, N_CORES, NODES_PER_CORE)
    return out, res


def kernel(**inputs):
    out, _ = _run(inputs, trace=False)
    return out


def kernel_profiled(**inputs):
    out, res = _run(inputs, trace=True)
    return out, res


# revision 30
# speedup vs baseline: 1.0185x; 1.0185x over previous
"""Trainium2 Bass kernel for nn_MeshNodeBlock (GNN message passing block).

reference semantics:
    agg = segment_sum(edge_features, src_indices, N)        # scatter-add
    x   = concat([node_features, agg], -1)
    h   = silu(x @ W1 + b1)
    y   = h @ W2 + b2
    y   = layer_norm(y) * gamma + beta
    out = y + node_features

Strategy (8 NeuronCores, SPMD, one NEFF):
  * Host graph-partitions nodes contiguously across cores (12800 node slots
    per core) and stable-sorts edges by destination node; each core receives
    exactly the edge rows destined for its nodes, grouped by 128-node tile
    and padded to a per-tile-position chunk count C_i (shared across cores
    so the SPMD program is uniform; pad rows are zero).
  * Device works fully in transposed space (features on partitions, nodes on
    free dim). Per 128-node tile the scatter-add is C_i PE matmuls
    aggT += edge_chunk.T @ onehot. One-hot blocks for a whole tile are built
    in one 2x-mode vector is_equal against a tiled-iota constant, with the
    local ids pre-expanded by a gpsimd broadcast copy.
  * MLP consumes aggT/nodeT directly: layer 1 -> hT_j slices, silu(+b1) on
    the scalar engine, layer 2 -> yT.
  * LayerNorm stats via matmuls whose lhsT is a block-diagonal 1/128 column
    (ONCB): group g's mean/mean-of-squares land on PSUM row g of a shared
    bank, accumulated over a block of groups. Stats post-processing
    (var, rstd=exp(-0.5*ln(var+eps))) runs once per block at full width,
    then rows bounce through a DRAM tile and DMA-broadcast back across
    partitions. Processing is phase-blocked to minimize ACT table switches.
  * Output written transposed in bf16; host transposes/casts back.
"""

import functools
from contextlib import ExitStack

import numpy as np
import ml_dtypes

import concourse.bass as bass
import concourse.tile as tile
from concourse import bacc, mybir
from concourse import bass_utils

BF16 = ml_dtypes.bfloat16
FP8 = ml_dtypes.float8_e4m3

N_NODES = 100000
D = 128
N_CORES = 8
P = 128
GROUP = 512              # nodes per group = 4 tiles
NODES_PER_CORE = 12800   # 25 groups
C_MAX = 8                # fallback chunk budget per tile (exact counts used)
NBLK = 2                 # phase blocks
INTERLEAVE_P3 = True    # interleave prev block's normalize into next phase1
EPS = 1e-5

AF = mybir.ActivationFunctionType
ALU = mybir.AluOpType
dt = mybir.dt


# --------------------------------------------------------------------------
# device kernel builder
# --------------------------------------------------------------------------

@functools.lru_cache(maxsize=4)
def _build(nodes_per_core: int, cis: tuple, n_cores: int, act: str = "silu"):
    assert nodes_per_core % GROUP == 0
    n_groups = nodes_per_core // GROUP
    tiles_per_core = nodes_per_core // P
    assert len(cis) == tiles_per_core
    coff = np.concatenate([[0], np.cumsum(cis)]).astype(int)
    ch = int(coff[-1])                   # total chunks per core
    cmaxt = int(max(cis))

    # phase blocks of groups (ACT table switches cost ~2.7us per set swap).
    # Asymmetric: big first block, small last block whose normalize tail is
    # all that remains after PE finishes.
    if n_groups >= 8:
        ntail = max(4, n_groups // 4)
        blocks = [list(range(0, n_groups - ntail)),
                  list(range(n_groups - ntail, n_groups))]
    else:
        blocks = [list(range(n_groups))]
    bmax = max(len(b) for b in blocks)

    nc = bacc.Bacc("TRN2", target_bir_lowering=False, debug=False,
                   enable_asserts=False, num_devices=n_cores)

    EB = nc.dram_tensor("eb", [P, ch * 128], dt.bfloat16, kind="ExternalInput").ap()
    OHD = nc.dram_tensor("ohd", [P, ch * 128], dt.float8e4,
                         kind="ExternalInput").ap()
    NTB = nc.dram_tensor("ntb", [P, nodes_per_core], dt.bfloat16,
                         kind="ExternalInput").ap()
    NPB = nc.dram_tensor("npb", [P, nodes_per_core], dt.bfloat16,
                         kind="ExternalInput").ap()
    W1P = nc.dram_tensor("w1p", [P, 1024], dt.bfloat16, kind="ExternalInput").ap()
    W2P = nc.dram_tensor("w2p", [P, 512], dt.bfloat16, kind="ExternalInput").ap()
    B1P = nc.dram_tensor("b1p", [P, 4], dt.float32, kind="ExternalInput").ap()
    B2P = nc.dram_tensor("b2p", [P, 1], dt.float32, kind="ExternalInput").ap()
    GAM = nc.dram_tensor("gam", [P, 1], dt.float32, kind="ExternalInput").ap()
    BET = nc.dram_tensor("bet", [P, 1], dt.float32, kind="ExternalInput").ap()
    ONB = nc.dram_tensor("onb", [P, bmax * 128], dt.bfloat16,
                         kind="ExternalInput").ap()
    OUT = nc.dram_tensor("out", [P, nodes_per_core], dt.bfloat16,
                         kind="ExternalOutput").ap()

    with tile.TileContext(nc) as tc:
        with ExitStack() as ctx:
            singles = ctx.enter_context(tc.tile_pool(name="singles", bufs=1))
            ebp = ctx.enter_context(tc.tile_pool(name="ebp", bufs=8))
            ohp = ctx.enter_context(tc.tile_pool(name="ohp", bufs=8))
            xtp = ctx.enter_context(tc.tile_pool(name="xtp", bufs=4))
            shp = ctx.enter_context(tc.tile_pool(name="shp", bufs=2))
            yp = ctx.enter_context(tc.tile_pool(name="yp", bufs=n_groups + 2))
            npp = ctx.enter_context(tc.tile_pool(name="npp", bufs=n_groups + 2))
            zp = ctx.enter_context(tc.tile_pool(name="zp", bufs=8))
            stp = ctx.enter_context(tc.tile_pool(name="stp", bufs=2))
            psagg = ctx.enter_context(tc.tile_pool(name="psagg", bufs=2, space="PSUM"))
            psh = ctx.enter_context(tc.tile_pool(name="psh", bufs=3, space="PSUM"))
            psy = ctx.enter_context(tc.tile_pool(name="psy", bufs=1, space="PSUM"))
            psst = ctx.enter_context(tc.tile_pool(name="psst", bufs=1, space="PSUM"))
            drp = ctx.enter_context(tc.tile_pool(name="drp", bufs=2, space="DRAM"))

            def load_const(name, src, shape, dtyp):
                t = singles.tile(shape, dtyp, tag=name)
                nc.sync.dma_start(out=t[:], in_=src)
                return t

            w1 = load_const("w1", W1P, [P, 1024], dt.bfloat16)
            w2 = load_const("w2", W2P, [P, 512], dt.bfloat16)
            b1 = load_const("b1", B1P, [P, 4], dt.float32)
            b2 = load_const("b2", B2P, [P, 1], dt.float32)
            gam = load_const("gam", GAM, [P, 1], dt.float32)
            bet = load_const("bet", BET, [P, 1], dt.float32)
            onb = load_const("onb", ONB, [P, bmax * 128], dt.bfloat16)
            eps = singles.tile([P, 1], dt.float32, tag="eps")
            nc.vector.memset(eps[:], EPS)

            y_tiles = {}
            npb_tiles = {}
            _last_stats = []

            xta_tiles = {}
            xtn_tiles = {}

            def phase1(block, bi, interleave=None):
                bsz = len(block)
                mu_ps = psst.tile([P, GROUP], dt.float32, tag="mups")
                m2_ps = psst.tile([P, GROUP], dt.float32, tag="m2ps")
                for gi, g in enumerate(block):
                    nsl = slice(g * GROUP, (g + 1) * GROUP)
                    xtn = xtp.tile([P, GROUP], dt.bfloat16, tag="xtn")
                    nc.sync.dma_start(out=xtn[:], in_=NTB[:, nsl])
                    xtn_tiles[g] = xtn
                    npbt = npp.tile([P, GROUP], dt.bfloat16, tag="npb")
                    nc.sync.dma_start(out=npbt[:], in_=NPB[:, nsl])
                    npb_tiles[g] = npbt

                    agg_ps = psagg.tile([P, GROUP], dt.float32, tag="agg")
                    for t4 in range(4):
                        ti = g * 4 + t4
                        cw = int(cis[ti]) * 128
                        o0 = int(coff[ti])
                        eb = ebp.tile([P, cmaxt * 128], dt.bfloat16, tag="eb")
                        nc.sync.dma_start(
                            out=eb[:, :cw], in_=EB[:, o0 * 128:o0 * 128 + cw])
                        oh = ohp.tile([P, cmaxt * 128], dt.float8e4, tag="oh")
                        nc.sync.dma_start(
                            out=oh[:, :cw], in_=OHD[:, o0 * 128:o0 * 128 + cw])
                        for c in range(int(cis[ti])):
                            nc.tensor.matmul(
                                out=agg_ps[:, t4 * 128:(t4 + 1) * 128],
                                lhsT=eb[:, c * 128:(c + 1) * 128],
                                rhs=oh[:, c * 128:(c + 1) * 128],
                                start=(c == 0), stop=(c == int(cis[ti]) - 1))
                    xta = xtp.tile([P, GROUP], dt.bfloat16, tag="xta")
                    if g % 2 == 0:
                        nc.scalar.activation(out=xta[:], in_=agg_ps[:], func=AF.Copy)
                    else:
                        nc.vector.tensor_copy(out=xta[:], in_=agg_ps[:])
                    sh_tiles = []
                    for j in range(4):
                        hps = psh.tile([P, GROUP], dt.float32, tag="hps")
                        nc.tensor.matmul(out=hps[:],
                                         lhsT=w1[:, j * 128:(j + 1) * 128],
                                         rhs=xtn[:], start=True, stop=False)
                        nc.tensor.matmul(
                            out=hps[:],
                            lhsT=w1[:, 512 + j * 128:512 + (j + 1) * 128],
                            rhs=xta[:], start=False, stop=True)
                        sh = shp.tile([P, GROUP], dt.bfloat16, tag=f"sh{j}")
                        if act == "silu":
                            nc.scalar.activation(out=sh[:], in_=hps[:],
                                                 func=AF.Silu,
                                                 bias=b1[:, j:j + 1], scale=1.0)
                        else:
                            sg = shp.tile([P, GROUP], dt.float32, tag=f"sg{j}")
                            nc.scalar.activation(out=sg[:], in_=hps[:],
                                                 func=AF.Sigmoid,
                                                 bias=b1[:, j:j + 1], scale=1.0)
                            u = shp.tile([P, GROUP], dt.float32, tag=f"u{j}")
                            nc.vector.tensor_scalar(
                                out=u[:], in0=hps[:], scalar1=b1[:, j:j + 1],
                                scalar2=None, op0=ALU.add)
                            nc.vector.tensor_tensor(out=sh[:], in0=u[:],
                                                    in1=sg[:], op=ALU.mult)
                        sh_tiles.append(sh)

                    yps = psy.tile([P, GROUP], dt.float32, tag="yps")
                    for j in range(4):
                        nc.tensor.matmul(out=yps[:],
                                         lhsT=w2[:, j * 128:(j + 1) * 128],
                                         rhs=sh_tiles[j][:],
                                         start=(j == 0), stop=(j == 3))
                    y = yp.tile([P, GROUP], dt.bfloat16, tag="y")
                    if g % 2 == 0:
                        nc.scalar.activation(out=y[:], in_=yps[:],
                                             func=AF.Identity,
                                             bias=b2[:, 0:1], scale=1.0)
                    else:
                        nc.vector.tensor_scalar(out=y[:], in0=yps[:],
                                                scalar1=b2[:, 0:1], scalar2=None,
                                                op0=ALU.add)
                    y_tiles[g] = y
                    y2 = zp.tile([P, GROUP], dt.bfloat16, tag="y2")
                    nc.vector.tensor_tensor(out=y2[:], in0=y[:], in1=y[:],
                                            op=ALU.mult)
                    onc_g = onb[:, gi * 128:(gi + 1) * 128]
                    nc.tensor.matmul(out=mu_ps[:], lhsT=onc_g, rhs=y[:],
                                     start=(gi == 0), stop=(gi == bsz - 1),
                                     skip_group_check=True)
                    nc.tensor.matmul(out=m2_ps[:], lhsT=onc_g, rhs=y2[:],
                                     start=(gi == 0), stop=(gi == bsz - 1),
                                     skip_group_check=True)
                    if INTERLEAVE_P3 and interleave and gi < len(interleave[0]):
                        phase3_group(interleave[0][gi], gi, interleave[1])
                if interleave:
                    for gi in range(len(block) if INTERLEAVE_P3 else 0,
                                    len(interleave[0])):
                        phase3_group(interleave[0][gi], gi, interleave[1])
                _last_stats.append((mu_ps, m2_ps))

            def phase2(block, bi, mu_ps, m2_ps):
                mu_bf = stp.tile([P, GROUP], dt.bfloat16, tag="mubf")
                nc.scalar.activation(out=mu_bf[:], in_=mu_ps[:], func=AF.Copy)
                m2_bf = stp.tile([P, GROUP], dt.bfloat16, tag="m2bf")
                nc.scalar.activation(out=m2_bf[:], in_=m2_ps[:], func=AF.Copy)
                musq = stp.tile([P, GROUP], dt.bfloat16, tag="musq")
                nc.scalar.square(out=musq[:], in_=mu_bf[:])
                var = stp.tile([P, GROUP], dt.bfloat16, tag="var")
                nc.vector.tensor_tensor(out=var[:], in0=m2_bf[:], in1=musq[:],
                                        op=ALU.subtract)
                lnv = stp.tile([P, GROUP], dt.bfloat16, tag="lnv")
                nc.scalar.activation(out=lnv[:], in_=var[:], func=AF.Ln,
                                     bias=eps[:, 0:1], scale=1.0)
                rstd = stp.tile([P, GROUP], dt.bfloat16, tag="rstd")
                nc.scalar.activation(out=rstd[:], in_=lnv[:], func=AF.Exp,
                                     bias=0.0, scale=-0.5)
                bounce = drp.tile([len(block), 1024], dt.bfloat16, tag="bounce")
                nc.sync.dma_start(out=bounce[:, 0:512],
                                  in_=mu_bf[0:len(block), :])
                nc.sync.dma_start(out=bounce[:, 512:1024],
                                  in_=rstd[0:len(block), :])
                return bounce

            def phase3_group(g, gi, bounce):
                    nsl = slice(g * GROUP, (g + 1) * GROUP)
                    mubc = zp.tile([P, GROUP], dt.bfloat16, tag="mubc")
                    bsl = bounce[gi:gi + 1, 0:512]
                    nc.sync.dma_start(out=mubc[:], in_=bass.AP(
                        tensor=bsl.tensor, offset=bsl.offset,
                        ap=[[0, P], bsl.ap[1]]))
                    rbc = zp.tile([P, GROUP], dt.bfloat16, tag="rbc")
                    bsl2 = bounce[gi:gi + 1, 512:1024]
                    nc.sync.dma_start(out=rbc[:], in_=bass.AP(
                        tensor=bsl2.tensor, offset=bsl2.offset,
                        ap=[[0, P], bsl2.ap[1]]))
                    y = y_tiles.pop(g)
                    npbt = npb_tiles.pop(g)
                    za = zp.tile([P, GROUP], dt.bfloat16, tag="za")
                    nc.vector.tensor_tensor(out=za[:], in0=y[:], in1=mubc[:],
                                            op=ALU.subtract)
                    zb = zp.tile([P, GROUP], dt.bfloat16, tag="zb")
                    nc.vector.tensor_tensor(out=zb[:], in0=za[:], in1=rbc[:],
                                            op=ALU.mult)
                    zc = zp.tile([P, GROUP], dt.bfloat16, tag="zc")
                    nc.vector.tensor_scalar(out=zc[:], in0=zb[:],
                                            scalar1=gam[:, 0:1],
                                            scalar2=bet[:, 0:1],
                                            op0=ALU.mult, op1=ALU.add)
                    of = zp.tile([P, GROUP], dt.bfloat16, tag="of")
                    nc.vector.tensor_tensor(out=of[:], in0=zc[:], in1=npbt[:],
                                            op=ALU.add)
                    nc.sync.dma_start(out=OUT[:, nsl], in_=of[:])

            # emission: P1(b) [with P3(b-1) interleaved between groups],
            # P2(b); final P3 for the last block.
            prev = None
            for bi, block in enumerate(blocks):
                phase1(block, bi, interleave=prev)
                mu_ps, m2_ps = _last_stats.pop()
                bounce = phase2(block, bi, mu_ps, m2_ps)
                prev = (block, bounce)
            for gi, g in enumerate(prev[0]):
                phase3_group(g, gi, prev[1])

    nc.compile()
    return nc


# --------------------------------------------------------------------------
# host-side sharding / packing
# --------------------------------------------------------------------------

def _preprocess(inputs, n_cores, nodes_per_core):
    nf = np.ascontiguousarray(np.asarray(inputs["node_features"], np.float32))
    ef = np.ascontiguousarray(np.asarray(inputs["edge_features"], np.float32))
    src = np.asarray(inputs["src_indices"]).astype(np.int64)
    W1 = np.asarray(inputs["W1"], np.float32)
    b1 = np.asarray(inputs["b1"], np.float32)
    W2 = np.asarray(inputs["W2"], np.float32)
    b2 = np.asarray(inputs["b2"], np.float32)
    gam = np.asarray(inputs["ln_gamma"], np.float32)
    bet = np.asarray(inputs["ln_beta"], np.float32)

    n_nodes, d = nf.shape
    n_edges = ef.shape[0]
    tiles_per_core = nodes_per_core // P
    n_groups = nodes_per_core // GROUP
    if n_groups >= 8:
        bmax = n_groups - max(4, n_groups // 4)
    else:
        bmax = n_groups

    order = np.argsort(src, kind="stable")
    snode = src[order]
    core = snode // nodes_per_core
    tile_in_core = (snode % nodes_per_core) // P
    lid = snode % P
    pt = core * tiles_per_core + tile_in_core
    counts = np.bincount(pt, minlength=n_cores * tiles_per_core)
    # per-position chunk counts, shared across cores (SPMD uniformity)
    ccounts = np.ceil(counts.reshape(n_cores, tiles_per_core) / P).astype(int)
    cis = np.maximum(ccounts.max(axis=0), 1)
    coff = np.concatenate([[0], np.cumsum(cis)]).astype(int)
    ch = int(coff[-1])
    cmaxt = int(cis.max())

    starts = np.zeros(n_cores * tiles_per_core, np.int64)
    np.cumsum(counts[:-1], out=starts[1:])
    rank = np.arange(n_edges, dtype=np.int64) - starts[pt]
    chunk = rank // P
    p = rank % P
    cg = coff[tile_in_core] + chunk
    row = core * (P * ch) + p * ch + cg

    ebuf = np.zeros((n_cores * P * ch, d), np.float32)
    ebuf[row] = ef[order]
    EBa = ebuf.reshape(n_cores, P, ch * d).astype(BF16)
    ohbuf = np.zeros((n_cores * P * ch, 128), FP8)
    ohbuf[row, lid] = 1.0
    OHa = ohbuf.reshape(n_cores, P, ch * 128)

    nfp = np.zeros((n_cores * nodes_per_core, d), np.float32)
    nfp[:n_nodes] = nf
    NTBa = np.ascontiguousarray(
        nfp.reshape(n_cores, nodes_per_core, d).transpose(0, 2, 1)).astype(BF16)
    nfp[:n_nodes] = nf + bet[None, :]
    nfp[n_nodes:] = bet[None, :]
    NPBa = np.ascontiguousarray(
        nfp.reshape(n_cores, nodes_per_core, d).transpose(0, 2, 1)).astype(BF16)

    W1P = np.ascontiguousarray(
        W1.reshape(2, P, 4, P).transpose(1, 0, 2, 3).reshape(P, 1024)).astype(BF16)
    W2P = np.ascontiguousarray(
        W2.reshape(4, P, P).transpose(1, 0, 2).reshape(P, 512)).astype(BF16)
    B1P = np.ascontiguousarray(b1.reshape(4, P).T)
    B2P = np.ascontiguousarray(b2.reshape(P, 1))
    GAMP = np.ascontiguousarray(gam.reshape(P, 1))
    # beta is folded into NPB; device beta input stays zero
    BETP = np.zeros((P, 1), np.float32)
    ONB = np.zeros((P, bmax * 128), np.float32)
    for g in range(bmax):
        ONB[:, g * 128 + g] = 1.0 / P
    ONB = ONB.astype(BF16)

    in_maps = []
    for k in range(n_cores):
        in_maps.append({
            "eb": EBa[k], "ohd": OHa[k], "ntb": NTBa[k], "npb": NPBa[k],
            "w1p": W1P, "w2p": W2P, "b1p": B1P, "b2p": B2P,
            "gam": GAMP, "bet": BETP, "onb": ONB,
        })
    return in_maps, tuple(int(c) for c in cis)


def _assemble(results, n_nodes, n_cores, nodes_per_core):
    outs = np.stack([np.asarray(r["out"]) for r in results])
    full = outs.astype(np.float32).transpose(0, 2, 1).reshape(
        n_cores * nodes_per_core, -1)
    return np.ascontiguousarray(full[:n_nodes])


# --------------------------------------------------------------------------
# public entry point
# --------------------------------------------------------------------------

ACT_MODE = "silu"

_AXON_SO = "/opt/axon/libaxon_pjrt.so"


def _ensure_ntff_hook():
    """Provide antenv.axon_hooks + register the ctypes NTFF profile hook
    (the agent image's antenv lacks axon_hooks, so boot degraded silently)."""
    import sys
    import types
    import ctypes
    import contextlib
    import os

    try:
        from antenv.axon_hooks import get_axon_ntff_profile_hook  # noqa: F401
        return
    except ImportError:
        pass
    import antenv

    m = types.ModuleType("antenv.axon_hooks")
    m._hook = None

    def set_axon_ntff_profile_hook(h):
        m._hook = h

    def get_axon_ntff_profile_hook():
        return m._hook

    m.set_axon_ntff_profile_hook = set_axon_ntff_profile_hook
    m.get_axon_ntff_profile_hook = get_axon_ntff_profile_hook
    sys.modules["antenv.axon_hooks"] = m
    antenv.axon_hooks = m

    if not os.path.exists(_AXON_SO):
        return
    lib = ctypes.CDLL(_AXON_SO)
    if not hasattr(lib, "axon_start_nrt_profile"):
        return
    lib.axon_start_nrt_profile.argtypes = [ctypes.POINTER(ctypes.c_int64),
                                           ctypes.c_size_t]
    lib.axon_start_nrt_profile.restype = ctypes.c_int64
    lib.axon_stop_nrt_profile.argtypes = [ctypes.c_char_p]
    lib.axon_stop_nrt_profile.restype = ctypes.c_int64

    @contextlib.contextmanager
    def _hook(output_dir, device_ids):
        import jax

        jax.devices()
        if device_ids:
            ids = (ctypes.c_int64 * len(device_ids))(*device_ids)
            rc = lib.axon_start_nrt_profile(ids, len(device_ids))
        else:
            rc = lib.axon_start_nrt_profile(None, 0)
        if rc != 0:
            raise RuntimeError(f"axon_start_nrt_profile rc={rc}")
        try:
            yield
        finally:
            n = lib.axon_stop_nrt_profile(str(output_dir).encode())
            if n < 0:
                raise RuntimeError(f"axon_stop_nrt_profile rc={n}")
            if n == 0:
                print("WARNING: NTFF capture wrote no files")

    m._hook = _hook


def _run(inputs, trace=False):
    if trace:
        _ensure_ntff_hook()
    n_nodes = np.asarray(inputs["node_features"]).shape[0]
    in_maps, cis = _preprocess(inputs, N_CORES, NODES_PER_CORE)
    nc = _build(NODES_PER_CORE, cis, N_CORES, ACT_MODE)
    res = bass_utils.run_bass_kernel_spmd(
        nc, in_maps, core_ids=list(range(N_CORES)), trace=trace)
    out = _assemble(res.results, n_nodes, N_CORES, NODES_PER_CORE)
    return out, res


def kernel(**inputs):
    out, _ = _run(inputs, trace=False)
    return out


def kernel_profiled(**inputs):
    out, res = _run(inputs, trace=True)
    return out, res


# revision 31
# speedup vs baseline: 1.0559x; 1.0367x over previous
"""Trainium2 Bass kernel for nn_MeshNodeBlock (GNN message passing block).

reference semantics:
    agg = segment_sum(edge_features, src_indices, N)        # scatter-add
    x   = concat([node_features, agg], -1)
    h   = silu(x @ W1 + b1)
    y   = h @ W2 + b2
    y   = layer_norm(y) * gamma + beta
    out = y + node_features

Strategy (8 NeuronCores, SPMD, one NEFF):
  * Host graph-partitions nodes contiguously across cores (12800 node slots
    per core) and stable-sorts edges by destination node; each core receives
    exactly the edge rows destined for its nodes, grouped by 128-node tile
    and padded to a per-tile-position chunk count C_i (shared across cores
    so the SPMD program is uniform; pad rows are zero).
  * Device works fully in transposed space (features on partitions, nodes on
    free dim). Per 128-node tile the scatter-add is C_i PE matmuls
    aggT += edge_chunk.T @ onehot. One-hot blocks for a whole tile are built
    in one 2x-mode vector is_equal against a tiled-iota constant, with the
    local ids pre-expanded by a gpsimd broadcast copy.
  * MLP consumes aggT/nodeT directly: layer 1 -> hT_j slices, silu(+b1) on
    the scalar engine, layer 2 -> yT.
  * LayerNorm stats via matmuls whose lhsT is a block-diagonal 1/128 column
    (ONCB): group g's mean/mean-of-squares land on PSUM row g of a shared
    bank, accumulated over a block of groups. Stats post-processing
    (var, rstd=exp(-0.5*ln(var+eps))) runs once per block at full width,
    then rows bounce through a DRAM tile and DMA-broadcast back across
    partitions. Processing is phase-blocked to minimize ACT table switches.
  * Output written transposed in bf16; host transposes/casts back.
"""

import functools
from contextlib import ExitStack

import numpy as np
import ml_dtypes

import concourse.bass as bass
import concourse.tile as tile
from concourse import bacc, mybir
from concourse import bass_utils

BF16 = ml_dtypes.bfloat16
FP8 = ml_dtypes.float8_e4m3

N_NODES = 100000
D = 128
N_CORES = 8
P = 128
GROUP = 512              # nodes per group = 4 tiles
NODES_PER_CORE = 12800   # 25 groups
C_MAX = 8                # fallback chunk budget per tile (exact counts used)
NBLK = 2                 # phase blocks
INTERLEAVE_P3 = True    # interleave prev block's normalize into next phase1
EPS = 1e-5

AF = mybir.ActivationFunctionType
ALU = mybir.AluOpType
dt = mybir.dt


# --------------------------------------------------------------------------
# device kernel builder
# --------------------------------------------------------------------------

@functools.lru_cache(maxsize=4)
def _build(nodes_per_core: int, cis: tuple, n_cores: int, act: str = "silu"):
    assert nodes_per_core % GROUP == 0
    n_groups = nodes_per_core // GROUP
    tiles_per_core = nodes_per_core // P
    assert len(cis) == tiles_per_core
    coff = np.concatenate([[0], np.cumsum(cis)]).astype(int)
    ch = int(coff[-1])                   # total chunks per core
    cmaxt = int(max(cis))

    # phase blocks of groups (ACT table switches cost ~2.7us per set swap).
    # Asymmetric: big first block, small last block whose normalize tail is
    # all that remains after PE finishes.
    if n_groups >= 8:
        ntail = max(4, n_groups // 4)
        blocks = [list(range(0, n_groups - ntail)),
                  list(range(n_groups - ntail, n_groups))]
    else:
        blocks = [list(range(n_groups))]
    bmax = max(len(b) for b in blocks)

    nc = bacc.Bacc("TRN2", target_bir_lowering=False, debug=False,
                   enable_asserts=False, num_devices=n_cores)

    EB = nc.dram_tensor("eb", [P, ch * 128], dt.bfloat16, kind="ExternalInput").ap()
    OHD = nc.dram_tensor("ohd", [P, ch * 128], dt.float8e4,
                         kind="ExternalInput").ap()
    NTB = nc.dram_tensor("ntb", [P, nodes_per_core], dt.bfloat16,
                         kind="ExternalInput").ap()
    W1P = nc.dram_tensor("w1p", [P, 1024], dt.bfloat16, kind="ExternalInput").ap()
    W2P = nc.dram_tensor("w2p", [P, 512], dt.bfloat16, kind="ExternalInput").ap()
    B1P = nc.dram_tensor("b1p", [P, 4], dt.float32, kind="ExternalInput").ap()
    B2P = nc.dram_tensor("b2p", [P, 1], dt.float32, kind="ExternalInput").ap()
    GAM = nc.dram_tensor("gam", [P, 1], dt.float32, kind="ExternalInput").ap()
    BET = nc.dram_tensor("bet", [P, 1], dt.float32, kind="ExternalInput").ap()
    ONB = nc.dram_tensor("onb", [P, bmax * 128], dt.bfloat16,
                         kind="ExternalInput").ap()
    OUT = nc.dram_tensor("out", [P, nodes_per_core], dt.bfloat16,
                         kind="ExternalOutput").ap()

    with tile.TileContext(nc) as tc:
        with ExitStack() as ctx:
            singles = ctx.enter_context(tc.tile_pool(name="singles", bufs=1))
            ebp = ctx.enter_context(tc.tile_pool(name="ebp", bufs=8))
            ohp = ctx.enter_context(tc.tile_pool(name="ohp", bufs=8))
            xtp = ctx.enter_context(tc.tile_pool(name="xtp", bufs=n_groups + 2))
            shp = ctx.enter_context(tc.tile_pool(name="shp", bufs=2))
            yp = ctx.enter_context(tc.tile_pool(name="yp", bufs=n_groups + 2))
            y2p = ctx.enter_context(tc.tile_pool(name="y2p", bufs=bmax + 2))
            zp = ctx.enter_context(tc.tile_pool(name="zp", bufs=8))
            stp = ctx.enter_context(tc.tile_pool(name="stp", bufs=2))
            psagg = ctx.enter_context(tc.tile_pool(name="psagg", bufs=2, space="PSUM"))
            psh = ctx.enter_context(tc.tile_pool(name="psh", bufs=3, space="PSUM"))
            psy = ctx.enter_context(tc.tile_pool(name="psy", bufs=1, space="PSUM"))
            psst = ctx.enter_context(tc.tile_pool(name="psst", bufs=1, space="PSUM"))
            drp = ctx.enter_context(tc.tile_pool(name="drp", bufs=2, space="DRAM"))

            def load_const(name, src, shape, dtyp):
                t = singles.tile(shape, dtyp, tag=name)
                nc.sync.dma_start(out=t[:], in_=src)
                return t

            w1 = load_const("w1", W1P, [P, 1024], dt.bfloat16)
            w2 = load_const("w2", W2P, [P, 512], dt.bfloat16)
            b1 = load_const("b1", B1P, [P, 4], dt.float32)
            b2 = load_const("b2", B2P, [P, 1], dt.float32)
            gam = load_const("gam", GAM, [P, 1], dt.float32)
            bet = load_const("bet", BET, [P, 1], dt.float32)
            onb = load_const("onb", ONB, [P, bmax * 128], dt.bfloat16)
            eps = singles.tile([P, 1], dt.float32, tag="eps")
            nc.vector.memset(eps[:], EPS)

            y_tiles = {}
            y2_tiles = {}
            _last_stats = []

            xta_tiles = {}
            xtn_tiles = {}

            def phase1(block, bi, interleave=None):
                bsz = len(block)
                mu_ps = psst.tile([P, GROUP], dt.float32, tag="mups")
                m2_ps = psst.tile([P, GROUP], dt.float32, tag="m2ps")
                for gi, g in enumerate(block):
                    nsl = slice(g * GROUP, (g + 1) * GROUP)
                    xtn = xtp.tile([P, GROUP], dt.bfloat16, tag="xtn")
                    nc.sync.dma_start(out=xtn[:], in_=NTB[:, nsl])
                    xtn_tiles[g] = xtn

                    agg_ps = psagg.tile([P, GROUP], dt.float32, tag="agg")
                    for t4 in range(4):
                        ti = g * 4 + t4
                        cw = int(cis[ti]) * 128
                        o0 = int(coff[ti])
                        eb = ebp.tile([P, cmaxt * 128], dt.bfloat16, tag="eb")
                        nc.sync.dma_start(
                            out=eb[:, :cw], in_=EB[:, o0 * 128:o0 * 128 + cw])
                        oh = ohp.tile([P, cmaxt * 128], dt.float8e4, tag="oh")
                        nc.sync.dma_start(
                            out=oh[:, :cw], in_=OHD[:, o0 * 128:o0 * 128 + cw])
                        for c in range(int(cis[ti])):
                            nc.tensor.matmul(
                                out=agg_ps[:, t4 * 128:(t4 + 1) * 128],
                                lhsT=eb[:, c * 128:(c + 1) * 128],
                                rhs=oh[:, c * 128:(c + 1) * 128],
                                start=(c == 0), stop=(c == int(cis[ti]) - 1))
                    xta = xtp.tile([P, GROUP], dt.bfloat16, tag="xta")
                    if g % 2 == 0:
                        nc.scalar.activation(out=xta[:], in_=agg_ps[:], func=AF.Copy)
                    else:
                        nc.vector.tensor_copy(out=xta[:], in_=agg_ps[:])
                    sh_tiles = []
                    for j in range(4):
                        hps = psh.tile([P, GROUP], dt.float32, tag="hps")
                        nc.tensor.matmul(out=hps[:],
                                         lhsT=w1[:, j * 128:(j + 1) * 128],
                                         rhs=xtn[:], start=True, stop=False)
                        nc.tensor.matmul(
                            out=hps[:],
                            lhsT=w1[:, 512 + j * 128:512 + (j + 1) * 128],
                            rhs=xta[:], start=False, stop=True)
                        sh = shp.tile([P, GROUP], dt.bfloat16, tag=f"sh{j}")
                        if act == "silu":
                            nc.scalar.activation(out=sh[:], in_=hps[:],
                                                 func=AF.Silu,
                                                 bias=b1[:, j:j + 1], scale=1.0)
                        else:
                            sg = shp.tile([P, GROUP], dt.float32, tag=f"sg{j}")
                            nc.scalar.activation(out=sg[:], in_=hps[:],
                                                 func=AF.Sigmoid,
                                                 bias=b1[:, j:j + 1], scale=1.0)
                            u = shp.tile([P, GROUP], dt.float32, tag=f"u{j}")
                            nc.vector.tensor_scalar(
                                out=u[:], in0=hps[:], scalar1=b1[:, j:j + 1],
                                scalar2=None, op0=ALU.add)
                            nc.vector.tensor_tensor(out=sh[:], in0=u[:],
                                                    in1=sg[:], op=ALU.mult)
                        sh_tiles.append(sh)

                    yps = psy.tile([P, GROUP], dt.float32, tag="yps")
                    for j in range(4):
                        nc.tensor.matmul(out=yps[:],
                                         lhsT=w2[:, j * 128:(j + 1) * 128],
                                         rhs=sh_tiles[j][:],
                                         start=(j == 0), stop=(j == 3))
                    y = yp.tile([P, GROUP], dt.bfloat16, tag="y")
                    nc.vector.tensor_scalar(out=y[:], in0=yps[:],
                                            scalar1=b2[:, 0:1], scalar2=None,
                                            op0=ALU.add)
                    y_tiles[g] = y
                    y2 = y2p.tile([P, GROUP], dt.bfloat16, tag="y2")
                    nc.vector.tensor_tensor(out=y2[:], in0=y[:], in1=y[:],
                                            op=ALU.mult)
                    y2_tiles[g] = y2
                # block-end stats burst (keeps stats matmuls off the
                # per-group PE critical path)
                for gi, g in enumerate(block):
                    onc_g = onb[:, gi * 128:(gi + 1) * 128]
                    nc.tensor.matmul(out=mu_ps[:], lhsT=onc_g,
                                     rhs=y_tiles[g][:],
                                     start=(gi == 0), stop=(gi == bsz - 1),
                                     skip_group_check=True)
                    nc.tensor.matmul(out=m2_ps[:], lhsT=onc_g,
                                     rhs=y2_tiles.pop(g)[:],
                                     start=(gi == 0), stop=(gi == bsz - 1),
                                     skip_group_check=True)
                _last_stats.append((mu_ps, m2_ps))

            def phase2(block, bi, mu_ps, m2_ps):
                mu_bf = stp.tile([P, GROUP], dt.bfloat16, tag="mubf")
                nc.scalar.activation(out=mu_bf[:], in_=mu_ps[:], func=AF.Copy)
                m2_bf = stp.tile([P, GROUP], dt.bfloat16, tag="m2bf")
                nc.scalar.activation(out=m2_bf[:], in_=m2_ps[:], func=AF.Copy)
                musq = stp.tile([P, GROUP], dt.bfloat16, tag="musq")
                nc.scalar.square(out=musq[:], in_=mu_bf[:])
                var = stp.tile([P, GROUP], dt.bfloat16, tag="var")
                nc.vector.tensor_tensor(out=var[:], in0=m2_bf[:], in1=musq[:],
                                        op=ALU.subtract)
                lnv = stp.tile([P, GROUP], dt.bfloat16, tag="lnv")
                nc.scalar.activation(out=lnv[:], in_=var[:], func=AF.Ln,
                                     bias=eps[:, 0:1], scale=1.0)
                rstd = stp.tile([P, GROUP], dt.bfloat16, tag="rstd")
                nc.scalar.activation(out=rstd[:], in_=lnv[:], func=AF.Exp,
                                     bias=0.0, scale=-0.5)
                bounce = drp.tile([len(block), 1024], dt.bfloat16, tag="bounce")
                nc.sync.dma_start(out=bounce[:, 0:512],
                                  in_=mu_bf[0:len(block), :])
                nc.sync.dma_start(out=bounce[:, 512:1024],
                                  in_=rstd[0:len(block), :])
                return bounce

            def phase3_group(g, gi, bounce):
                    nsl = slice(g * GROUP, (g + 1) * GROUP)
                    mr = zp.tile([P, 1024], dt.bfloat16, tag="mr")
                    bsl = bounce[gi:gi + 1, 0:1024]
                    nc.sync.dma_start(out=mr[:], in_=bass.AP(
                        tensor=bsl.tensor, offset=bsl.offset,
                        ap=[[0, P], bsl.ap[1]]))
                    y = y_tiles.pop(g)
                    xtn = xtn_tiles.pop(g)
                    za = zp.tile([P, GROUP], dt.bfloat16, tag="za")
                    nc.vector.tensor_tensor(out=za[:], in0=y[:],
                                            in1=mr[:, 0:512], op=ALU.subtract)
                    zb = zp.tile([P, GROUP], dt.bfloat16, tag="zb")
                    nc.vector.tensor_tensor(out=zb[:], in0=za[:],
                                            in1=mr[:, 512:1024], op=ALU.mult)
                    zc = zp.tile([P, GROUP], dt.bfloat16, tag="zc")
                    nc.vector.tensor_scalar(out=zc[:], in0=zb[:],
                                            scalar1=gam[:, 0:1],
                                            scalar2=bet[:, 0:1],
                                            op0=ALU.mult, op1=ALU.add)
                    of = zp.tile([P, GROUP], dt.bfloat16, tag="of")
                    nc.vector.tensor_tensor(out=of[:], in0=zc[:], in1=xtn[:],
                                            op=ALU.add)
                    nc.sync.dma_start(out=OUT[:, nsl], in_=of[:])

            # emission: P1(b) P2(b) P3(b). P3 is DVE+DMA-only; with the
            # stats burst at block end, P1(b+1)'s PE work has no DVE
            # dependencies that queue behind P3(b)'s chains.
            for bi, block in enumerate(blocks):
                phase1(block, bi)
                mu_ps, m2_ps = _last_stats.pop()
                bounce = phase2(block, bi, mu_ps, m2_ps)
                for gi, g in enumerate(block):
                    phase3_group(g, gi, bounce)

    nc.compile()
    return nc


# --------------------------------------------------------------------------
# host-side sharding / packing
# --------------------------------------------------------------------------

def _preprocess(inputs, n_cores, nodes_per_core):
    nf = np.ascontiguousarray(np.asarray(inputs["node_features"], np.float32))
    ef = np.ascontiguousarray(np.asarray(inputs["edge_features"], np.float32))
    src = np.asarray(inputs["src_indices"]).astype(np.int64)
    W1 = np.asarray(inputs["W1"], np.float32)
    b1 = np.asarray(inputs["b1"], np.float32)
    W2 = np.asarray(inputs["W2"], np.float32)
    b2 = np.asarray(inputs["b2"], np.float32)
    gam = np.asarray(inputs["ln_gamma"], np.float32)
    bet = np.asarray(inputs["ln_beta"], np.float32)

    n_nodes, d = nf.shape
    n_edges = ef.shape[0]
    tiles_per_core = nodes_per_core // P
    n_groups = nodes_per_core // GROUP
    if n_groups >= 8:
        bmax = n_groups - max(4, n_groups // 4)
    else:
        bmax = n_groups

    order = np.argsort(src, kind="stable")
    snode = src[order]
    core = snode // nodes_per_core
    tile_in_core = (snode % nodes_per_core) // P
    lid = snode % P
    pt = core * tiles_per_core + tile_in_core
    counts = np.bincount(pt, minlength=n_cores * tiles_per_core)
    # per-position chunk counts, shared across cores (SPMD uniformity)
    ccounts = np.ceil(counts.reshape(n_cores, tiles_per_core) / P).astype(int)
    cis = np.maximum(ccounts.max(axis=0), 1)
    coff = np.concatenate([[0], np.cumsum(cis)]).astype(int)
    ch = int(coff[-1])
    cmaxt = int(cis.max())

    starts = np.zeros(n_cores * tiles_per_core, np.int64)
    np.cumsum(counts[:-1], out=starts[1:])
    rank = np.arange(n_edges, dtype=np.int64) - starts[pt]
    chunk = rank // P
    p = rank % P
    cg = coff[tile_in_core] + chunk
    row = core * (P * ch) + p * ch + cg

    ebuf = np.zeros((n_cores * P * ch, d), np.float32)
    ebuf[row] = ef[order]
    EBa = ebuf.reshape(n_cores, P, ch * d).astype(BF16)
    ohbuf = np.zeros((n_cores * P * ch, 128), FP8)
    ohbuf[row, lid] = 1.0
    OHa = ohbuf.reshape(n_cores, P, ch * 128)

    nfp = np.zeros((n_cores * nodes_per_core, d), np.float32)
    nfp[:n_nodes] = nf
    NTBa = np.ascontiguousarray(
        nfp.reshape(n_cores, nodes_per_core, d).transpose(0, 2, 1)).astype(BF16)

    W1P = np.ascontiguousarray(
        W1.reshape(2, P, 4, P).transpose(1, 0, 2, 3).reshape(P, 1024)).astype(BF16)
    W2P = np.ascontiguousarray(
        W2.reshape(4, P, P).transpose(1, 0, 2).reshape(P, 512)).astype(BF16)
    B1P = np.ascontiguousarray(b1.reshape(4, P).T)
    B2P = np.ascontiguousarray(b2.reshape(P, 1))
    GAMP = np.ascontiguousarray(gam.reshape(P, 1))
    BETP = np.ascontiguousarray(bet.reshape(P, 1))
    ONB = np.zeros((P, bmax * 128), np.float32)
    for g in range(bmax):
        ONB[:, g * 128 + g] = 1.0 / P
    ONB = ONB.astype(BF16)

    in_maps = []
    for k in range(n_cores):
        in_maps.append({
            "eb": EBa[k], "ohd": OHa[k], "ntb": NTBa[k],
            "w1p": W1P, "w2p": W2P, "b1p": B1P, "b2p": B2P,
            "gam": GAMP, "bet": BETP, "onb": ONB,
        })
    return in_maps, tuple(int(c) for c in cis)


def _assemble(results, n_nodes, n_cores, nodes_per_core):
    outs = np.stack([np.asarray(r["out"]) for r in results])
    full = outs.astype(np.float32).transpose(0, 2, 1).reshape(
        n_cores * nodes_per_core, -1)
    return np.ascontiguousarray(full[:n_nodes])


# --------------------------------------------------------------------------
# public entry point
# --------------------------------------------------------------------------

ACT_MODE = "silu"

_AXON_SO = "/opt/axon/libaxon_pjrt.so"


def _ensure_ntff_hook():
    """Provide antenv.axon_hooks + register the ctypes NTFF profile hook
    (the agent image's antenv lacks axon_hooks, so boot degraded silently)."""
    import sys
    import types
    import ctypes
    import contextlib
    import os

    try:
        from antenv.axon_hooks import get_axon_ntff_profile_hook  # noqa: F401
        return
    except ImportError:
        pass
    import antenv

    m = types.ModuleType("antenv.axon_hooks")
    m._hook = None

    def set_axon_ntff_profile_hook(h):
        m._hook = h

    def get_axon_ntff_profile_hook():
        return m._hook

    m.set_axon_ntff_profile_hook = set_axon_ntff_profile_hook
    m.get_axon_ntff_profile_hook = get_axon_ntff_profile_hook
    sys.modules["antenv.axon_hooks"] = m
    antenv.axon_hooks = m

    if not os.path.exists(_AXON_SO):
        return
    lib = ctypes.CDLL(_AXON_SO)
    if not hasattr(lib, "axon_start_nrt_profile"):
        return
    lib.axon_start_nrt_profile.argtypes = [ctypes.POINTER(ctypes.c_int64),
                                           ctypes.c_size_t]
    lib.axon_start_nrt_profile.restype = ctypes.c_int64
    lib.axon_stop_nrt_profile.argtypes = [ctypes.c_char_p]
    lib.axon_stop_nrt_profile.restype = ctypes.c_int64

    @contextlib.contextmanager
    def _hook(output_dir, device_ids):
        import jax

        jax.devices()
        if device_ids:
            ids = (ctypes.c_int64 * len(device_ids))(*device_ids)
            rc = lib.axon_start_nrt_profile(ids, len(device_ids))
        else:
            rc = lib.axon_start_nrt_profile(None, 0)
        if rc != 0:
            raise RuntimeError(f"axon_start_nrt_profile rc={rc}")
        try:
            yield
        finally:
            n = lib.axon_stop_nrt_profile(str(output_dir).encode())
            if n < 0:
                raise RuntimeError(f"axon_stop_nrt_profile rc={n}")
            if n == 0:
                print("WARNING: NTFF capture wrote no files")

    m._hook = _hook


def _run(inputs, trace=False):
    if trace:
        _ensure_ntff_hook()
    n_nodes = np.asarray(inputs["node_features"]).shape[0]
    in_maps, cis = _preprocess(inputs, N_CORES, NODES_PER_CORE)
    nc = _build(NODES_PER_CORE, cis, N_CORES, ACT_MODE)
    res = bass_utils.run_bass_kernel_spmd(
        nc, in_maps, core_ids=list(range(N_CORES)), trace=trace)
    out = _assemble(res.results, n_nodes, N_CORES, NODES_PER_CORE)
    return out, res


def kernel(**inputs):
    out, _ = _run(inputs, trace=False)
    return out


def kernel_profiled(**inputs):
    out, res = _run(inputs, trace=True)
    return out, res


# revision 33
# speedup vs baseline: 1.1773x; 1.1150x over previous
"""Trainium2 Bass kernel for nn_MeshNodeBlock (GNN message passing block).

reference semantics:
    agg = segment_sum(edge_features, src_indices, N)        # scatter-add
    x   = concat([node_features, agg], -1)
    h   = silu(x @ W1 + b1)
    y   = h @ W2 + b2
    y   = layer_norm(y) * gamma + beta
    out = y + node_features

Strategy (8 NeuronCores, SPMD, one NEFF):
  * Host graph-partitions nodes contiguously across cores (12800 node slots
    per core) and stable-sorts edges by destination node; each core receives
    exactly the edge rows destined for its nodes, grouped by 128-node tile
    and padded to a per-tile-position chunk count C_i (shared across cores
    so the SPMD program is uniform; pad rows are zero).
  * Device works fully in transposed space (features on partitions, nodes on
    free dim). Per 128-node tile the scatter-add is C_i PE matmuls
    aggT += edge_chunk.T @ onehot. One-hot blocks for a whole tile are built
    in one 2x-mode vector is_equal against a tiled-iota constant, with the
    local ids pre-expanded by a gpsimd broadcast copy.
  * MLP consumes aggT/nodeT directly: layer 1 -> hT_j slices, silu(+b1) on
    the scalar engine, layer 2 -> yT.
  * LayerNorm stats via matmuls whose lhsT is a block-diagonal 1/128 column
    (ONCB): group g's mean/mean-of-squares land on PSUM row g of a shared
    bank, accumulated over a block of groups. Stats post-processing
    (var, rstd=exp(-0.5*ln(var+eps))) runs once per block at full width,
    then rows bounce through a DRAM tile and DMA-broadcast back across
    partitions. Processing is phase-blocked to minimize ACT table switches.
  * Output written transposed in bf16; host transposes/casts back.
"""

import functools
from contextlib import ExitStack

import numpy as np
import ml_dtypes

import concourse.bass as bass
import concourse.tile as tile
from concourse import bacc, mybir
from concourse import bass_utils

BF16 = ml_dtypes.bfloat16
FP8 = ml_dtypes.float8_e4m3

N_NODES = 100000
D = 128
N_CORES = 8
P = 128
GROUP = 512              # nodes per group = 4 tiles
NODES_PER_CORE = 12800   # 25 groups
C_MAX = 8                # fallback chunk budget per tile (exact counts used)
NBLK = 2                 # phase blocks
INTERLEAVE_P3 = True    # interleave prev block's normalize into next phase1
EPS = 1e-5

AF = mybir.ActivationFunctionType
ALU = mybir.AluOpType
dt = mybir.dt


# --------------------------------------------------------------------------
# device kernel builder
# --------------------------------------------------------------------------

@functools.lru_cache(maxsize=4)
def _build(nodes_per_core: int, cis: tuple, n_cores: int, act: str = "silu"):
    assert nodes_per_core % GROUP == 0
    n_groups = nodes_per_core // GROUP
    tiles_per_core = nodes_per_core // P
    assert len(cis) == tiles_per_core
    coff = np.concatenate([[0], np.cumsum(cis)]).astype(int)
    ch = int(coff[-1])                   # total chunks per core
    cmaxt = int(max(cis))
    gbytes_max = max(
        (int(coff[gg * 4 + 4]) - int(coff[gg * 4])) * 384
        for gg in range(nodes_per_core // GROUP))

    # phase blocks of groups (ACT table switches cost ~2.7us per set swap).
    # Asymmetric: big first block, small last block whose normalize tail is
    # all that remains after PE finishes.
    if n_groups >= 8:
        ntail = max(4, n_groups // 4)
        blocks = [list(range(0, n_groups - ntail)),
                  list(range(n_groups - ntail, n_groups))]
    else:
        blocks = [list(range(n_groups))]
    bmax = max(len(b) for b in blocks)

    nc = bacc.Bacc("TRN2", target_bir_lowering=False, debug=False,
                   enable_asserts=False, num_devices=n_cores)

    PK = nc.dram_tensor("pk", [P, ch * 384], dt.uint8, kind="ExternalInput").ap()
    NTB = nc.dram_tensor("ntb", [P, nodes_per_core], dt.bfloat16,
                         kind="ExternalInput").ap()
    W1P = nc.dram_tensor("w1p", [P, 1024], dt.bfloat16, kind="ExternalInput").ap()
    W2P = nc.dram_tensor("w2p", [P, 512], dt.bfloat16, kind="ExternalInput").ap()
    B1P = nc.dram_tensor("b1p", [P, 4], dt.float32, kind="ExternalInput").ap()
    B2P = nc.dram_tensor("b2p", [P, 1], dt.float32, kind="ExternalInput").ap()
    GAM = nc.dram_tensor("gam", [P, 1], dt.float32, kind="ExternalInput").ap()
    BET = nc.dram_tensor("bet", [P, 1], dt.float32, kind="ExternalInput").ap()
    ONB = nc.dram_tensor("onb", [P, bmax * 128], dt.bfloat16,
                         kind="ExternalInput").ap()
    OUT = nc.dram_tensor("out", [P, nodes_per_core], dt.bfloat16,
                         kind="ExternalOutput").ap()

    with tile.TileContext(nc) as tc:
        with ExitStack() as ctx:
            singles = ctx.enter_context(tc.tile_pool(name="singles", bufs=1))
            ebp = ctx.enter_context(tc.tile_pool(name="ebp", bufs=4))
            xtp = ctx.enter_context(tc.tile_pool(name="xtp", bufs=n_groups + 2))
            xap = ctx.enter_context(tc.tile_pool(name="xap", bufs=4))
            shp = ctx.enter_context(tc.tile_pool(name="shp", bufs=2))
            yp = ctx.enter_context(tc.tile_pool(name="yp", bufs=n_groups + 2))
            y2p = ctx.enter_context(tc.tile_pool(name="y2p", bufs=bmax + 2))
            zp = ctx.enter_context(tc.tile_pool(name="zp", bufs=4))
            stp = ctx.enter_context(tc.tile_pool(name="stp", bufs=1))
            psagg = ctx.enter_context(tc.tile_pool(name="psagg", bufs=2, space="PSUM"))
            psh = ctx.enter_context(tc.tile_pool(name="psh", bufs=3, space="PSUM"))
            psy = ctx.enter_context(tc.tile_pool(name="psy", bufs=1, space="PSUM"))
            psst = ctx.enter_context(tc.tile_pool(name="psst", bufs=1, space="PSUM"))
            drp = ctx.enter_context(tc.tile_pool(name="drp", bufs=2, space="DRAM"))

            def load_const(name, src, shape, dtyp):
                t = singles.tile(shape, dtyp, tag=name)
                nc.sync.dma_start(out=t[:], in_=src)
                return t

            w1 = load_const("w1", W1P, [P, 1024], dt.bfloat16)
            w2 = load_const("w2", W2P, [P, 512], dt.bfloat16)
            b1 = load_const("b1", B1P, [P, 4], dt.float32)
            b2 = load_const("b2", B2P, [P, 1], dt.float32)
            gam = load_const("gam", GAM, [P, 1], dt.float32)
            bet = load_const("bet", BET, [P, 1], dt.float32)
            onb = load_const("onb", ONB, [P, bmax * 128], dt.bfloat16)
            eps = singles.tile([P, 1], dt.float32, tag="eps")
            nc.vector.memset(eps[:], EPS)

            y_tiles = {}
            y2_tiles = {}
            _last_stats = []

            xta_tiles = {}
            xtn_tiles = {}

            def phase1(block, bi, interleave=None):
                bsz = len(block)
                mu_ps = psst.tile([P, GROUP], dt.float32, tag="mups")
                m2_ps = psst.tile([P, GROUP], dt.float32, tag="m2ps")
                for gi, g in enumerate(block):
                    nsl = slice(g * GROUP, (g + 1) * GROUP)
                    xtn = xtp.tile([P, GROUP], dt.bfloat16, tag="xtn")
                    nc.sync.dma_start(out=xtn[:], in_=NTB[:, nsl])
                    xtn_tiles[g] = xtn

                    agg_ps = psagg.tile([P, GROUP], dt.float32, tag="agg")
                    g0 = int(coff[g * 4])
                    gbytes = (int(coff[g * 4 + 4]) - g0) * 384
                    pk = ebp.tile([P, gbytes_max], dt.uint8, tag="pk")
                    nc.sync.dma_start(out=pk[:, :gbytes],
                                      in_=PK[:, g0 * 384:g0 * 384 + gbytes])
                    for t4 in range(4):
                        ti = g * 4 + t4
                        ci = int(cis[ti])
                        toff = (int(coff[ti]) - g0) * 384
                        ebv = pk[:, toff:toff + ci * 256].bitcast(dt.bfloat16)
                        ohv = pk[:, toff + ci * 256:toff + ci * 384].bitcast(
                            dt.float8e4)
                        for c in range(ci):
                            nc.tensor.matmul(
                                out=agg_ps[:, t4 * 128:(t4 + 1) * 128],
                                lhsT=ebv[:, c * 128:(c + 1) * 128],
                                rhs=ohv[:, c * 128:(c + 1) * 128],
                                start=(c == 0), stop=(c == ci - 1))
                    xta = xap.tile([P, GROUP], dt.bfloat16, tag="xta")
                    if g % 2 == 0:
                        nc.scalar.activation(out=xta[:], in_=agg_ps[:], func=AF.Copy)
                    else:
                        nc.vector.tensor_copy(out=xta[:], in_=agg_ps[:])
                    sh_tiles = []
                    for j in range(4):
                        hps = psh.tile([P, GROUP], dt.float32, tag="hps")
                        nc.tensor.matmul(out=hps[:],
                                         lhsT=w1[:, j * 128:(j + 1) * 128],
                                         rhs=xtn[:], start=True, stop=False)
                        nc.tensor.matmul(
                            out=hps[:],
                            lhsT=w1[:, 512 + j * 128:512 + (j + 1) * 128],
                            rhs=xta[:], start=False, stop=True)
                        sh = shp.tile([P, GROUP], dt.bfloat16, tag=f"sh{j}")
                        if act == "silu":
                            nc.scalar.activation(out=sh[:], in_=hps[:],
                                                 func=AF.Silu,
                                                 bias=b1[:, j:j + 1], scale=1.0)
                        else:
                            sg = shp.tile([P, GROUP], dt.float32, tag=f"sg{j}")
                            nc.scalar.activation(out=sg[:], in_=hps[:],
                                                 func=AF.Sigmoid,
                                                 bias=b1[:, j:j + 1], scale=1.0)
                            u = shp.tile([P, GROUP], dt.float32, tag=f"u{j}")
                            nc.vector.tensor_scalar(
                                out=u[:], in0=hps[:], scalar1=b1[:, j:j + 1],
                                scalar2=None, op0=ALU.add)
                            nc.vector.tensor_tensor(out=sh[:], in0=u[:],
                                                    in1=sg[:], op=ALU.mult)
                        sh_tiles.append(sh)

                    yps = psy.tile([P, GROUP], dt.float32, tag="yps")
                    for j in range(4):
                        nc.tensor.matmul(out=yps[:],
                                         lhsT=w2[:, j * 128:(j + 1) * 128],
                                         rhs=sh_tiles[j][:],
                                         start=(j == 0), stop=(j == 3))
                    y = yp.tile([P, GROUP], dt.bfloat16, tag="y")
                    nc.vector.tensor_scalar(out=y[:], in0=yps[:],
                                            scalar1=b2[:, 0:1], scalar2=None,
                                            op0=ALU.add)
                    y_tiles[g] = y
                    y2 = y2p.tile([P, GROUP], dt.bfloat16, tag="y2")
                    nc.vector.tensor_tensor(out=y2[:], in0=y[:], in1=y[:],
                                            op=ALU.mult)
                    y2_tiles[g] = y2
                # block-end stats burst (keeps stats matmuls off the
                # per-group PE critical path)
                for gi, g in enumerate(block):
                    onc_g = onb[:, gi * 128:(gi + 1) * 128]
                    nc.tensor.matmul(out=mu_ps[:], lhsT=onc_g,
                                     rhs=y_tiles[g][:],
                                     start=(gi == 0), stop=(gi == bsz - 1),
                                     skip_group_check=True)
                    nc.tensor.matmul(out=m2_ps[:], lhsT=onc_g,
                                     rhs=y2_tiles.pop(g)[:],
                                     start=(gi == 0), stop=(gi == bsz - 1),
                                     skip_group_check=True)
                _last_stats.append((mu_ps, m2_ps))

            def phase2(block, bi, mu_ps, m2_ps):
                mu_bf = stp.tile([P, GROUP], dt.bfloat16, tag="mubf")
                nc.scalar.activation(out=mu_bf[:], in_=mu_ps[:], func=AF.Copy)
                m2_bf = stp.tile([P, GROUP], dt.bfloat16, tag="m2bf")
                nc.scalar.activation(out=m2_bf[:], in_=m2_ps[:], func=AF.Copy)
                musq = stp.tile([P, GROUP], dt.bfloat16, tag="musq")
                nc.scalar.square(out=musq[:], in_=mu_bf[:])
                var = stp.tile([P, GROUP], dt.bfloat16, tag="var")
                nc.vector.tensor_tensor(out=var[:], in0=m2_bf[:], in1=musq[:],
                                        op=ALU.subtract)
                lnv = stp.tile([P, GROUP], dt.bfloat16, tag="lnv")
                nc.scalar.activation(out=lnv[:], in_=var[:], func=AF.Ln,
                                     bias=eps[:, 0:1], scale=1.0)
                rstd = stp.tile([P, GROUP], dt.bfloat16, tag="rstd")
                nc.scalar.activation(out=rstd[:], in_=lnv[:], func=AF.Exp,
                                     bias=0.0, scale=-0.5)
                bounce = drp.tile([len(block), 1024], dt.bfloat16, tag="bounce")
                nc.sync.dma_start(out=bounce[:, 0:512],
                                  in_=mu_bf[0:len(block), :])
                nc.sync.dma_start(out=bounce[:, 512:1024],
                                  in_=rstd[0:len(block), :])
                return bounce

            def phase3_group(g, gi, bounce):
                    nsl = slice(g * GROUP, (g + 1) * GROUP)
                    mr = zp.tile([P, 1024], dt.bfloat16, tag="mr")
                    bsl = bounce[gi:gi + 1, 0:1024]
                    nc.scalar.dma_start(out=mr[:], in_=bass.AP(
                        tensor=bsl.tensor, offset=bsl.offset,
                        ap=[[0, P], bsl.ap[1]]))
                    y = y_tiles.pop(g)
                    xtn = xtn_tiles.pop(g)
                    za = zp.tile([P, GROUP], dt.bfloat16, tag="za")
                    nc.vector.tensor_tensor(out=za[:], in0=y[:],
                                            in1=mr[:, 0:512], op=ALU.subtract)
                    zb = zp.tile([P, GROUP], dt.bfloat16, tag="zb")
                    nc.vector.tensor_tensor(out=zb[:], in0=za[:],
                                            in1=mr[:, 512:1024], op=ALU.mult)
                    zc = zp.tile([P, GROUP], dt.bfloat16, tag="zc")
                    nc.vector.tensor_scalar(out=zc[:], in0=zb[:],
                                            scalar1=gam[:, 0:1],
                                            scalar2=bet[:, 0:1],
                                            op0=ALU.mult, op1=ALU.add)
                    of = zp.tile([P, GROUP], dt.bfloat16, tag="of")
                    nc.vector.tensor_tensor(out=of[:], in0=zc[:], in1=xtn[:],
                                            op=ALU.add)
                    nc.scalar.dma_start(out=OUT[:, nsl], in_=of[:])

            # emission: P1(b) P2(b) P3(b). P3 is DVE+DMA-only; with the
            # stats burst at block end, P1(b+1)'s PE work has no DVE
            # dependencies that queue behind P3(b)'s chains.
            for bi, block in enumerate(blocks):
                phase1(block, bi)
                mu_ps, m2_ps = _last_stats.pop()
                bounce = phase2(block, bi, mu_ps, m2_ps)
                for gi, g in enumerate(block):
                    phase3_group(g, gi, bounce)

    nc.compile()
    return nc


# --------------------------------------------------------------------------
# host-side sharding / packing
# --------------------------------------------------------------------------

def _preprocess(inputs, n_cores, nodes_per_core):
    nf = np.ascontiguousarray(np.asarray(inputs["node_features"], np.float32))
    ef = np.ascontiguousarray(np.asarray(inputs["edge_features"], np.float32))
    src = np.asarray(inputs["src_indices"]).astype(np.int64)
    W1 = np.asarray(inputs["W1"], np.float32)
    b1 = np.asarray(inputs["b1"], np.float32)
    W2 = np.asarray(inputs["W2"], np.float32)
    b2 = np.asarray(inputs["b2"], np.float32)
    gam = np.asarray(inputs["ln_gamma"], np.float32)
    bet = np.asarray(inputs["ln_beta"], np.float32)

    n_nodes, d = nf.shape
    n_edges = ef.shape[0]
    tiles_per_core = nodes_per_core // P
    n_groups = nodes_per_core // GROUP
    if n_groups >= 8:
        bmax = n_groups - max(4, n_groups // 4)
    else:
        bmax = n_groups

    order = np.argsort(src, kind="stable")
    snode = src[order]
    core = snode // nodes_per_core
    tile_in_core = (snode % nodes_per_core) // P
    lid = snode % P
    pt = core * tiles_per_core + tile_in_core
    counts = np.bincount(pt, minlength=n_cores * tiles_per_core)
    # per-position chunk counts, shared across cores (SPMD uniformity)
    ccounts = np.ceil(counts.reshape(n_cores, tiles_per_core) / P).astype(int)
    cis = np.maximum(ccounts.max(axis=0), 1)
    coff = np.concatenate([[0], np.cumsum(cis)]).astype(int)
    ch = int(coff[-1])
    cmaxt = int(cis.max())

    starts = np.zeros(n_cores * tiles_per_core, np.int64)
    np.cumsum(counts[:-1], out=starts[1:])
    rank = np.arange(n_edges, dtype=np.int64) - starts[pt]
    chunk = rank // P
    p = rank % P
    cg = coff[tile_in_core] + chunk
    row = core * (P * ch) + p * ch + cg

    ebuf = np.zeros((n_cores * P * ch, d), np.float32)
    ebuf[row] = ef[order]
    EB8 = ebuf.reshape(n_cores, P, ch * d).astype(BF16).view(np.uint8)
    ohbuf = np.zeros((n_cores * P * ch, 128), FP8)
    ohbuf[row, lid] = 1.0
    OH8 = ohbuf.reshape(n_cores, P, ch * 128).view(np.uint8)
    parts = []
    for ti in range(tiles_per_core):
        a, b = int(coff[ti]), int(coff[ti + 1])
        parts.append(EB8[:, :, a * 256:b * 256])
        parts.append(OH8[:, :, a * 128:b * 128])
    PKa = np.ascontiguousarray(np.concatenate(parts, axis=2))

    nfp = np.zeros((n_cores * nodes_per_core, d), np.float32)
    nfp[:n_nodes] = nf
    NTBa = np.ascontiguousarray(
        nfp.reshape(n_cores, nodes_per_core, d).transpose(0, 2, 1)).astype(BF16)

    W1P = np.ascontiguousarray(
        W1.reshape(2, P, 4, P).transpose(1, 0, 2, 3).reshape(P, 1024)).astype(BF16)
    W2P = np.ascontiguousarray(
        W2.reshape(4, P, P).transpose(1, 0, 2).reshape(P, 512)).astype(BF16)
    B1P = np.ascontiguousarray(b1.reshape(4, P).T)
    B2P = np.ascontiguousarray(b2.reshape(P, 1))
    GAMP = np.ascontiguousarray(gam.reshape(P, 1))
    BETP = np.ascontiguousarray(bet.reshape(P, 1))
    ONB = np.zeros((P, bmax * 128), np.float32)
    for g in range(bmax):
        ONB[:, g * 128 + g] = 1.0 / P
    ONB = ONB.astype(BF16)

    in_maps = []
    for k in range(n_cores):
        in_maps.append({
            "pk": PKa[k], "ntb": NTBa[k],
            "w1p": W1P, "w2p": W2P, "b1p": B1P, "b2p": B2P,
            "gam": GAMP, "bet": BETP, "onb": ONB,
        })
    return in_maps, tuple(int(c) for c in cis)


def _assemble(results, n_nodes, n_cores, nodes_per_core):
    outs = np.stack([np.asarray(r["out"]) for r in results])
    full = outs.astype(np.float32).transpose(0, 2, 1).reshape(
        n_cores * nodes_per_core, -1)
    return np.ascontiguousarray(full[:n_nodes])


# --------------------------------------------------------------------------
# public entry point
# --------------------------------------------------------------------------

ACT_MODE = "silu"

_AXON_SO = "/opt/axon/libaxon_pjrt.so"


def _ensure_ntff_hook():
    """Provide antenv.axon_hooks + register the ctypes NTFF profile hook
    (the agent image's antenv lacks axon_hooks, so boot degraded silently)."""
    import sys
    import types
    import ctypes
    import contextlib
    import os

    try:
        from antenv.axon_hooks import get_axon_ntff_profile_hook  # noqa: F401
        return
    except ImportError:
        pass
    import antenv

    m = types.ModuleType("antenv.axon_hooks")
    m._hook = None

    def set_axon_ntff_profile_hook(h):
        m._hook = h

    def get_axon_ntff_profile_hook():
        return m._hook

    m.set_axon_ntff_profile_hook = set_axon_ntff_profile_hook
    m.get_axon_ntff_profile_hook = get_axon_ntff_profile_hook
    sys.modules["antenv.axon_hooks"] = m
    antenv.axon_hooks = m

    if not os.path.exists(_AXON_SO):
        return
    lib = ctypes.CDLL(_AXON_SO)
    if not hasattr(lib, "axon_start_nrt_profile"):
        return
    lib.axon_start_nrt_profile.argtypes = [ctypes.POINTER(ctypes.c_int64),
                                           ctypes.c_size_t]
    lib.axon_start_nrt_profile.restype = ctypes.c_int64
    lib.axon_stop_nrt_profile.argtypes = [ctypes.c_char_p]
    lib.axon_stop_nrt_profile.restype = ctypes.c_int64

    @contextlib.contextmanager
    def _hook(output_dir, device_ids):
        import jax

        jax.devices()
        if device_ids:
            ids = (ctypes.c_int64 * len(device_ids))(*device_ids)
            rc = lib.axon_start_nrt_profile(ids, len(device_ids))
        else:
            rc = lib.axon_start_nrt_profile(None, 0)
        if rc != 0:
            raise RuntimeError(f"axon_start_nrt_profile rc={rc}")
        try:
            yield
        finally:
            n = lib.axon_stop_nrt_profile(str(output_dir).encode())
            if n < 0:
                raise RuntimeError(f"axon_stop_nrt_profile rc={n}")
            if n == 0:
                print("WARNING: NTFF capture wrote no files")

    m._hook = _hook


def _run(inputs, trace=False):
    if trace:
        _ensure_ntff_hook()
    n_nodes = np.asarray(inputs["node_features"]).shape[0]
    in_maps, cis = _preprocess(inputs, N_CORES, NODES_PER_CORE)
    nc = _build(NODES_PER_CORE, cis, N_CORES, ACT_MODE)
    res = bass_utils.run_bass_kernel_spmd(
        nc, in_maps, core_ids=list(range(N_CORES)), trace=trace)
    out = _assemble(res.results, n_nodes, N_CORES, NODES_PER_CORE)
    return out, res


def kernel(**inputs):
    out, _ = _run(inputs, trace=False)
    return out


def kernel_profiled(**inputs):
    out, res = _run(inputs, trace=True)
    return out, res


# revision 34
# speedup vs baseline: 1.1861x; 1.0075x over previous
"""Trainium2 Bass kernel for nn_MeshNodeBlock (GNN message passing block).

reference semantics:
    agg = segment_sum(edge_features, src_indices, N)        # scatter-add
    x   = concat([node_features, agg], -1)
    h   = silu(x @ W1 + b1)
    y   = h @ W2 + b2
    y   = layer_norm(y) * gamma + beta
    out = y + node_features

Strategy (8 NeuronCores, SPMD, one NEFF):
  * Host graph-partitions nodes contiguously across cores (12800 node slots
    per core) and stable-sorts edges by destination node; each core receives
    exactly the edge rows destined for its nodes, grouped by 128-node tile
    and padded to a per-tile-position chunk count C_i (shared across cores
    so the SPMD program is uniform; pad rows are zero).
  * Device works fully in transposed space (features on partitions, nodes on
    free dim). Per 128-node tile the scatter-add is C_i PE matmuls
    aggT += edge_chunk.T @ onehot. One-hot blocks for a whole tile are built
    in one 2x-mode vector is_equal against a tiled-iota constant, with the
    local ids pre-expanded by a gpsimd broadcast copy.
  * MLP consumes aggT/nodeT directly: layer 1 -> hT_j slices, silu(+b1) on
    the scalar engine, layer 2 -> yT.
  * LayerNorm stats via matmuls whose lhsT is a block-diagonal 1/128 column
    (ONCB): group g's mean/mean-of-squares land on PSUM row g of a shared
    bank, accumulated over a block of groups. Stats post-processing
    (var, rstd=exp(-0.5*ln(var+eps))) runs once per block at full width,
    then rows bounce through a DRAM tile and DMA-broadcast back across
    partitions. Processing is phase-blocked to minimize ACT table switches.
  * Output written transposed in bf16; host transposes/casts back.
"""

import functools
from contextlib import ExitStack

import numpy as np
import ml_dtypes

import concourse.bass as bass
import concourse.tile as tile
from concourse import bacc, mybir
from concourse import bass_utils

BF16 = ml_dtypes.bfloat16
FP8 = ml_dtypes.float8_e4m3

N_NODES = 100000
D = 128
N_CORES = 8
P = 128
GROUP = 512              # nodes per group = 4 tiles
NODES_PER_CORE = 12800   # 25 groups
C_MAX = 8                # fallback chunk budget per tile (exact counts used)
NBLK = 2                 # phase blocks
INTERLEAVE_P3 = True    # interleave prev block's normalize into next phase1
EPS = 1e-5

AF = mybir.ActivationFunctionType
ALU = mybir.AluOpType
dt = mybir.dt


# --------------------------------------------------------------------------
# device kernel builder
# --------------------------------------------------------------------------

@functools.lru_cache(maxsize=4)
def _build(nodes_per_core: int, cis: tuple, n_cores: int, act: str = "silu"):
    assert nodes_per_core % GROUP == 0
    n_groups = nodes_per_core // GROUP
    tiles_per_core = nodes_per_core // P
    assert len(cis) == tiles_per_core
    coff = np.concatenate([[0], np.cumsum(cis)]).astype(int)
    ch = int(coff[-1])                   # total chunks per core
    cmaxt = int(max(cis))
    gbytes_max = max(
        (int(coff[gg * 4 + 4]) - int(coff[gg * 4])) * 384
        for gg in range(nodes_per_core // GROUP))

    # phase blocks of groups (ACT table switches cost ~2.7us per set swap).
    # Asymmetric: big first block, small last block whose normalize tail is
    # all that remains after PE finishes.
    if n_groups >= 8:
        ntail = max(4, n_groups // 4)
        blocks = [list(range(0, n_groups - ntail)),
                  list(range(n_groups - ntail, n_groups))]
    else:
        blocks = [list(range(n_groups))]
    bmax = max(len(b) for b in blocks)

    nc = bacc.Bacc("TRN2", target_bir_lowering=False, debug=False,
                   enable_asserts=False, num_devices=n_cores)

    PK = nc.dram_tensor("pk", [P, ch * 384], dt.uint8, kind="ExternalInput").ap()
    NTB = nc.dram_tensor("ntb", [P, nodes_per_core], dt.bfloat16,
                         kind="ExternalInput").ap()
    W1P = nc.dram_tensor("w1p", [P, 1024], dt.bfloat16, kind="ExternalInput").ap()
    W2P = nc.dram_tensor("w2p", [P, 512], dt.bfloat16, kind="ExternalInput").ap()
    B1P = nc.dram_tensor("b1p", [P, 4], dt.float32, kind="ExternalInput").ap()
    B2P = nc.dram_tensor("b2p", [P, 1], dt.float32, kind="ExternalInput").ap()
    GAM = nc.dram_tensor("gam", [P, 1], dt.float32, kind="ExternalInput").ap()
    BET = nc.dram_tensor("bet", [P, 1], dt.float32, kind="ExternalInput").ap()
    ONB = nc.dram_tensor("onb", [P, bmax * 128], dt.bfloat16,
                         kind="ExternalInput").ap()
    OUT = nc.dram_tensor("out", [P, nodes_per_core], dt.bfloat16,
                         kind="ExternalOutput").ap()

    with tile.TileContext(nc) as tc:
        with ExitStack() as ctx:
            singles = ctx.enter_context(tc.tile_pool(name="singles", bufs=1))
            ebp = ctx.enter_context(tc.tile_pool(name="ebp", bufs=4))
            xtp = ctx.enter_context(tc.tile_pool(name="xtp", bufs=n_groups + 2))
            xap = ctx.enter_context(tc.tile_pool(name="xap", bufs=4))
            shp = ctx.enter_context(tc.tile_pool(name="shp", bufs=2))
            yp = ctx.enter_context(tc.tile_pool(name="yp", bufs=n_groups + 2))
            y2p = ctx.enter_context(tc.tile_pool(name="y2p", bufs=bmax + 2))
            zp = ctx.enter_context(tc.tile_pool(name="zp", bufs=4))
            stp = ctx.enter_context(tc.tile_pool(name="stp", bufs=1))
            psagg = ctx.enter_context(tc.tile_pool(name="psagg", bufs=2, space="PSUM"))
            psh = ctx.enter_context(tc.tile_pool(name="psh", bufs=3, space="PSUM"))
            psy = ctx.enter_context(tc.tile_pool(name="psy", bufs=1, space="PSUM"))
            psst = ctx.enter_context(tc.tile_pool(name="psst", bufs=1, space="PSUM"))
            drp = ctx.enter_context(tc.tile_pool(name="drp", bufs=2, space="DRAM"))

            def load_const(name, src, shape, dtyp):
                t = singles.tile(shape, dtyp, tag=name)
                nc.sync.dma_start(out=t[:], in_=src)
                return t

            w1 = load_const("w1", W1P, [P, 1024], dt.bfloat16)
            w2 = load_const("w2", W2P, [P, 512], dt.bfloat16)
            b1 = load_const("b1", B1P, [P, 4], dt.float32)
            b2 = load_const("b2", B2P, [P, 1], dt.float32)
            gam = load_const("gam", GAM, [P, 1], dt.float32)
            bet = load_const("bet", BET, [P, 1], dt.float32)
            onb = load_const("onb", ONB, [P, bmax * 128], dt.bfloat16)
            eps = singles.tile([P, 1], dt.float32, tag="eps")
            nc.vector.memset(eps[:], EPS)

            y_tiles = {}
            y2_tiles = {}
            _last_stats = []

            xta_tiles = {}
            xtn_tiles = {}

            def phase1(block, bi, interleave=None):
                bsz = len(block)
                mu_ps = psst.tile([P, GROUP], dt.float32, tag="mups")
                m2_ps = psst.tile([P, GROUP], dt.float32, tag="m2ps")
                for gi, g in enumerate(block):
                    nsl = slice(g * GROUP, (g + 1) * GROUP)
                    xtn = xtp.tile([P, GROUP], dt.bfloat16, tag="xtn")
                    nc.sync.dma_start(out=xtn[:], in_=NTB[:, nsl])
                    xtn_tiles[g] = xtn

                    agg_ps = psagg.tile([P, GROUP], dt.float32, tag="agg")
                    g0 = int(coff[g * 4])
                    gbytes = (int(coff[g * 4 + 4]) - g0) * 384
                    pk = ebp.tile([P, gbytes_max], dt.uint8, tag="pk")
                    nc.sync.dma_start(out=pk[:, :gbytes],
                                      in_=PK[:, g0 * 384:g0 * 384 + gbytes])
                    for t4 in range(4):
                        ti = g * 4 + t4
                        ci = int(cis[ti])
                        toff = (int(coff[ti]) - g0) * 384
                        ebv = pk[:, toff:toff + ci * 256].bitcast(dt.bfloat16)
                        ohv = pk[:, toff + ci * 256:toff + ci * 384].bitcast(
                            dt.float8e4)
                        for c in range(ci):
                            nc.tensor.matmul(
                                out=agg_ps[:, t4 * 128:(t4 + 1) * 128],
                                lhsT=ebv[:, c * 128:(c + 1) * 128],
                                rhs=ohv[:, c * 128:(c + 1) * 128],
                                start=(c == 0), stop=(c == ci - 1))
                    xta = xap.tile([P, GROUP], dt.bfloat16, tag="xta")
                    if g % 2 == 0:
                        nc.scalar.activation(out=xta[:], in_=agg_ps[:], func=AF.Copy)
                    else:
                        nc.vector.tensor_copy(out=xta[:], in_=agg_ps[:])
                    sh_tiles = []
                    for j in range(4):
                        hps = psh.tile([P, GROUP], dt.float32, tag="hps")
                        nc.tensor.matmul(out=hps[:],
                                         lhsT=w1[:, j * 128:(j + 1) * 128],
                                         rhs=xtn[:], start=True, stop=False)
                        nc.tensor.matmul(
                            out=hps[:],
                            lhsT=w1[:, 512 + j * 128:512 + (j + 1) * 128],
                            rhs=xta[:], start=False, stop=True)
                        sh = shp.tile([P, GROUP], dt.bfloat16, tag=f"sh{j}")
                        if act == "silu":
                            nc.scalar.activation(out=sh[:], in_=hps[:],
                                                 func=AF.Silu,
                                                 bias=b1[:, j:j + 1], scale=1.0)
                        else:
                            sg = shp.tile([P, GROUP], dt.float32, tag=f"sg{j}")
                            nc.scalar.activation(out=sg[:], in_=hps[:],
                                                 func=AF.Sigmoid,
                                                 bias=b1[:, j:j + 1], scale=1.0)
                            u = shp.tile([P, GROUP], dt.float32, tag=f"u{j}")
                            nc.vector.tensor_scalar(
                                out=u[:], in0=hps[:], scalar1=b1[:, j:j + 1],
                                scalar2=None, op0=ALU.add)
                            nc.vector.tensor_tensor(out=sh[:], in0=u[:],
                                                    in1=sg[:], op=ALU.mult)
                        sh_tiles.append(sh)

                    yps = psy.tile([P, GROUP], dt.float32, tag="yps")
                    for j in range(4):
                        nc.tensor.matmul(out=yps[:],
                                         lhsT=w2[:, j * 128:(j + 1) * 128],
                                         rhs=sh_tiles[j][:],
                                         start=(j == 0), stop=(j == 3))
                    y = yp.tile([P, GROUP], dt.bfloat16, tag="y")
                    nc.vector.tensor_scalar(out=y[:], in0=yps[:],
                                            scalar1=b2[:, 0:1], scalar2=None,
                                            op0=ALU.add)
                    y_tiles[g] = y
                    y2 = y2p.tile([P, GROUP], dt.bfloat16, tag="y2")
                    nc.vector.tensor_tensor(out=y2[:], in0=y[:], in1=y[:],
                                            op=ALU.mult)
                    y2_tiles[g] = y2
                # block-end stats burst (keeps stats matmuls off the
                # per-group PE critical path)
                for gi, g in enumerate(block):
                    onc_g = onb[:, gi * 128:(gi + 1) * 128]
                    nc.tensor.matmul(out=mu_ps[:], lhsT=onc_g,
                                     rhs=y_tiles[g][:],
                                     start=(gi == 0), stop=(gi == bsz - 1),
                                     skip_group_check=True)
                    nc.tensor.matmul(out=m2_ps[:], lhsT=onc_g,
                                     rhs=y2_tiles.pop(g)[:],
                                     start=(gi == 0), stop=(gi == bsz - 1),
                                     skip_group_check=True)
                _last_stats.append((mu_ps, m2_ps))

            def phase2(block, bi, mu_ps, m2_ps):
                mu_bf = stp.tile([P, GROUP], dt.bfloat16, tag="mubf")
                nc.scalar.activation(out=mu_bf[:], in_=mu_ps[:], func=AF.Copy)
                m2_bf = stp.tile([P, GROUP], dt.bfloat16, tag="m2bf")
                nc.scalar.activation(out=m2_bf[:], in_=m2_ps[:], func=AF.Copy)
                musq = stp.tile([P, GROUP], dt.bfloat16, tag="musq")
                nc.scalar.square(out=musq[:], in_=mu_bf[:])
                var = stp.tile([P, GROUP], dt.bfloat16, tag="var")
                nc.vector.tensor_tensor(out=var[:], in0=m2_bf[:], in1=musq[:],
                                        op=ALU.subtract)
                lnv = stp.tile([P, GROUP], dt.bfloat16, tag="lnv")
                nc.scalar.activation(out=lnv[:], in_=var[:], func=AF.Ln,
                                     bias=eps[:, 0:1], scale=1.0)
                rstd = stp.tile([P, GROUP], dt.bfloat16, tag="rstd")
                nc.scalar.activation(out=rstd[:], in_=lnv[:], func=AF.Exp,
                                     bias=0.0, scale=-0.5)
                bounce = drp.tile([len(block), 1024], dt.bfloat16, tag="bounce")
                nc.scalar.dma_start(out=bounce[:, 0:512],
                                    in_=mu_bf[0:len(block), :])
                nc.scalar.dma_start(out=bounce[:, 512:1024],
                                    in_=rstd[0:len(block), :])
                return bounce

            def phase3_group(g, gi, bounce):
                    nsl = slice(g * GROUP, (g + 1) * GROUP)
                    mr = zp.tile([P, 1024], dt.bfloat16, tag="mr")
                    bsl = bounce[gi:gi + 1, 0:1024]
                    nc.scalar.dma_start(out=mr[:], in_=bass.AP(
                        tensor=bsl.tensor, offset=bsl.offset,
                        ap=[[0, P], bsl.ap[1]]))
                    y = y_tiles.pop(g)
                    xtn = xtn_tiles.pop(g)
                    za = zp.tile([P, GROUP], dt.bfloat16, tag="za")
                    nc.vector.tensor_tensor(out=za[:], in0=y[:],
                                            in1=mr[:, 0:512], op=ALU.subtract)
                    zb = zp.tile([P, GROUP], dt.bfloat16, tag="zb")
                    nc.vector.tensor_tensor(out=zb[:], in0=za[:],
                                            in1=mr[:, 512:1024], op=ALU.mult)
                    zc = zp.tile([P, GROUP], dt.bfloat16, tag="zc")
                    nc.vector.tensor_scalar(out=zc[:], in0=zb[:],
                                            scalar1=gam[:, 0:1],
                                            scalar2=bet[:, 0:1],
                                            op0=ALU.mult, op1=ALU.add)
                    of = zp.tile([P, GROUP], dt.bfloat16, tag="of")
                    nc.vector.tensor_tensor(out=of[:], in0=zc[:], in1=xtn[:],
                                            op=ALU.add)
                    nc.scalar.dma_start(out=OUT[:, nsl], in_=of[:])

            # emission: P1(b) P2(b) P3(b). P3 is DVE+DMA-only; with the
            # stats burst at block end, P1(b+1)'s PE work has no DVE
            # dependencies that queue behind P3(b)'s chains.
            for bi, block in enumerate(blocks):
                phase1(block, bi)
                mu_ps, m2_ps = _last_stats.pop()
                bounce = phase2(block, bi, mu_ps, m2_ps)
                for gi, g in enumerate(block):
                    phase3_group(g, gi, bounce)

    nc.compile()
    return nc


# --------------------------------------------------------------------------
# host-side sharding / packing
# --------------------------------------------------------------------------

def _preprocess(inputs, n_cores, nodes_per_core):
    nf = np.ascontiguousarray(np.asarray(inputs["node_features"], np.float32))
    ef = np.ascontiguousarray(np.asarray(inputs["edge_features"], np.float32))
    src = np.asarray(inputs["src_indices"]).astype(np.int64)
    W1 = np.asarray(inputs["W1"], np.float32)
    b1 = np.asarray(inputs["b1"], np.float32)
    W2 = np.asarray(inputs["W2"], np.float32)
    b2 = np.asarray(inputs["b2"], np.float32)
    gam = np.asarray(inputs["ln_gamma"], np.float32)
    bet = np.asarray(inputs["ln_beta"], np.float32)

    n_nodes, d = nf.shape
    n_edges = ef.shape[0]
    tiles_per_core = nodes_per_core // P
    n_groups = nodes_per_core // GROUP
    if n_groups >= 8:
        bmax = n_groups - max(4, n_groups // 4)
    else:
        bmax = n_groups

    order = np.argsort(src, kind="stable")
    snode = src[order]
    core = snode // nodes_per_core
    tile_in_core = (snode % nodes_per_core) // P
    lid = snode % P
    pt = core * tiles_per_core + tile_in_core
    counts = np.bincount(pt, minlength=n_cores * tiles_per_core)
    # per-position chunk counts, shared across cores (SPMD uniformity)
    ccounts = np.ceil(counts.reshape(n_cores, tiles_per_core) / P).astype(int)
    cis = np.maximum(ccounts.max(axis=0), 1)
    coff = np.concatenate([[0], np.cumsum(cis)]).astype(int)
    ch = int(coff[-1])
    cmaxt = int(cis.max())

    starts = np.zeros(n_cores * tiles_per_core, np.int64)
    np.cumsum(counts[:-1], out=starts[1:])
    rank = np.arange(n_edges, dtype=np.int64) - starts[pt]
    chunk = rank // P
    p = rank % P
    cg = coff[tile_in_core] + chunk
    row = core * (P * ch) + p * ch + cg

    ebuf = np.zeros((n_cores * P * ch, d), np.float32)
    ebuf[row] = ef[order]
    EB8 = ebuf.reshape(n_cores, P, ch * d).astype(BF16).view(np.uint8)
    ohbuf = np.zeros((n_cores * P * ch, 128), FP8)
    ohbuf[row, lid] = 1.0
    OH8 = ohbuf.reshape(n_cores, P, ch * 128).view(np.uint8)
    parts = []
    for ti in range(tiles_per_core):
        a, b = int(coff[ti]), int(coff[ti + 1])
        parts.append(EB8[:, :, a * 256:b * 256])
        parts.append(OH8[:, :, a * 128:b * 128])
    PKa = np.ascontiguousarray(np.concatenate(parts, axis=2))

    nfp = np.zeros((n_cores * nodes_per_core, d), np.float32)
    nfp[:n_nodes] = nf
    NTBa = np.ascontiguousarray(
        nfp.reshape(n_cores, nodes_per_core, d).transpose(0, 2, 1)).astype(BF16)

    W1P = np.ascontiguousarray(
        W1.reshape(2, P, 4, P).transpose(1, 0, 2, 3).reshape(P, 1024)).astype(BF16)
    W2P = np.ascontiguousarray(
        W2.reshape(4, P, P).transpose(1, 0, 2).reshape(P, 512)).astype(BF16)
    B1P = np.ascontiguousarray(b1.reshape(4, P).T)
    B2P = np.ascontiguousarray(b2.reshape(P, 1))
    GAMP = np.ascontiguousarray(gam.reshape(P, 1))
    BETP = np.ascontiguousarray(bet.reshape(P, 1))
    ONB = np.zeros((P, bmax * 128), np.float32)
    for g in range(bmax):
        ONB[:, g * 128 + g] = 1.0 / P
    ONB = ONB.astype(BF16)

    in_maps = []
    for k in range(n_cores):
        in_maps.append({
            "pk": PKa[k], "ntb": NTBa[k],
            "w1p": W1P, "w2p": W2P, "b1p": B1P, "b2p": B2P,
            "gam": GAMP, "bet": BETP, "onb": ONB,
        })
    return in_maps, tuple(int(c) for c in cis)


def _assemble(results, n_nodes, n_cores, nodes_per_core):
    outs = np.stack([np.asarray(r["out"]) for r in results])
    full = outs.astype(np.float32).transpose(0, 2, 1).reshape(
        n_cores * nodes_per_core, -1)
    return np.ascontiguousarray(full[:n_nodes])


# --------------------------------------------------------------------------
# public entry point
# --------------------------------------------------------------------------

ACT_MODE = "silu"

_AXON_SO = "/opt/axon/libaxon_pjrt.so"


def _ensure_ntff_hook():
    """Provide antenv.axon_hooks + register the ctypes NTFF profile hook
    (the agent image's antenv lacks axon_hooks, so boot degraded silently)."""
    import sys
    import types
    import ctypes
    import contextlib
    import os

    try:
        from antenv.axon_hooks import get_axon_ntff_profile_hook  # noqa: F401
        return
    except ImportError:
        pass
    import antenv

    m = types.ModuleType("antenv.axon_hooks")
    m._hook = None

    def set_axon_ntff_profile_hook(h):
        m._hook = h

    def get_axon_ntff_profile_hook():
        return m._hook

    m.set_axon_ntff_profile_hook = set_axon_ntff_profile_hook
    m.get_axon_ntff_profile_hook = get_axon_ntff_profile_hook
    sys.modules["antenv.axon_hooks"] = m
    antenv.axon_hooks = m

    if not os.path.exists(_AXON_SO):
        return
    lib = ctypes.CDLL(_AXON_SO)
    if not hasattr(lib, "axon_start_nrt_profile"):
        return
    lib.axon_start_nrt_profile.argtypes = [ctypes.POINTER(ctypes.c_int64),
                                           ctypes.c_size_t]
    lib.axon_start_nrt_profile.restype = ctypes.c_int64
    lib.axon_stop_nrt_profile.argtypes = [ctypes.c_char_p]
    lib.axon_stop_nrt_profile.restype = ctypes.c_int64

    @contextlib.contextmanager
    def _hook(output_dir, device_ids):
        import jax

        jax.devices()
        if device_ids:
            ids = (ctypes.c_int64 * len(device_ids))(*device_ids)
            rc = lib.axon_start_nrt_profile(ids, len(device_ids))
        else:
            rc = lib.axon_start_nrt_profile(None, 0)
        if rc != 0:
            raise RuntimeError(f"axon_start_nrt_profile rc={rc}")
        try:
            yield
        finally:
            n = lib.axon_stop_nrt_profile(str(output_dir).encode())
            if n < 0:
                raise RuntimeError(f"axon_stop_nrt_profile rc={n}")
            if n == 0:
                print("WARNING: NTFF capture wrote no files")

    m._hook = _hook


def _run(inputs, trace=False):
    if trace:
        _ensure_ntff_hook()
    n_nodes = np.asarray(inputs["node_features"]).shape[0]
    in_maps, cis = _preprocess(inputs, N_CORES, NODES_PER_CORE)
    nc = _build(NODES_PER_CORE, cis, N_CORES, ACT_MODE)
    res = bass_utils.run_bass_kernel_spmd(
        nc, in_maps, core_ids=list(range(N_CORES)), trace=trace)
    out = _assemble(res.results, n_nodes, N_CORES, NODES_PER_CORE)
    return out, res


def kernel(**inputs):
    out, _ = _run(inputs, trace=False)
    return out


def kernel_profiled(**inputs):
    out, res = _run(inputs, trace=True)
    return out, res


# revision 35
# speedup vs baseline: 1.1913x; 1.0044x over previous
"""Trainium2 Bass kernel for nn_MeshNodeBlock (GNN message passing block).

reference semantics:
    agg = segment_sum(edge_features, src_indices, N)        # scatter-add
    x   = concat([node_features, agg], -1)
    h   = silu(x @ W1 + b1)
    y   = h @ W2 + b2
    y   = layer_norm(y) * gamma + beta
    out = y + node_features

Strategy (8 NeuronCores, SPMD, one NEFF):
  * Host graph-partitions nodes contiguously across cores (12800 node slots
    per core) and stable-sorts edges by destination node; each core receives
    exactly the edge rows destined for its nodes, grouped by 128-node tile
    and padded to a per-tile-position chunk count C_i (shared across cores
    so the SPMD program is uniform; pad rows are zero).
  * Device works fully in transposed space (features on partitions, nodes on
    free dim). Per 128-node tile the scatter-add is C_i PE matmuls
    aggT += edge_chunk.T @ onehot. One-hot blocks for a whole tile are built
    in one 2x-mode vector is_equal against a tiled-iota constant, with the
    local ids pre-expanded by a gpsimd broadcast copy.
  * MLP consumes aggT/nodeT directly: layer 1 -> hT_j slices, silu(+b1) on
    the scalar engine, layer 2 -> yT.
  * LayerNorm stats via matmuls whose lhsT is a block-diagonal 1/128 column
    (ONCB): group g's mean/mean-of-squares land on PSUM row g of a shared
    bank, accumulated over a block of groups. Stats post-processing
    (var, rstd=exp(-0.5*ln(var+eps))) runs once per block at full width,
    then rows bounce through a DRAM tile and DMA-broadcast back across
    partitions. Processing is phase-blocked to minimize ACT table switches.
  * Output written transposed in bf16; host transposes/casts back.
"""

import functools
from contextlib import ExitStack

import numpy as np
import ml_dtypes

import concourse.bass as bass
import concourse.tile as tile
from concourse import bacc, mybir
from concourse import bass_utils

BF16 = ml_dtypes.bfloat16
FP8 = ml_dtypes.float8_e4m3

N_NODES = 100000
D = 128
N_CORES = 8
P = 128
GROUP = 512              # nodes per group = 4 tiles
NODES_PER_CORE = 12800   # 25 groups
C_MAX = 8                # fallback chunk budget per tile (exact counts used)
NBLK = 2                 # phase blocks
INTERLEAVE_P3 = True    # interleave prev block's normalize into next phase1
EPS = 1e-5

AF = mybir.ActivationFunctionType
ALU = mybir.AluOpType
dt = mybir.dt


# --------------------------------------------------------------------------
# device kernel builder
# --------------------------------------------------------------------------

@functools.lru_cache(maxsize=4)
def _build(nodes_per_core: int, cis: tuple, n_cores: int, act: str = "silu"):
    assert nodes_per_core % GROUP == 0
    n_groups = nodes_per_core // GROUP
    tiles_per_core = nodes_per_core // P
    assert len(cis) == tiles_per_core
    coff = np.concatenate([[0], np.cumsum(cis)]).astype(int)
    ch = int(coff[-1])                   # total chunks per core
    cmaxt = int(max(cis))
    gbytes_max = max(
        (int(coff[gg * 4 + 4]) - int(coff[gg * 4])) * 384
        for gg in range(nodes_per_core // GROUP))

    # phase blocks of groups (ACT table switches cost ~2.7us per set swap).
    # Asymmetric: big first block, small last block whose normalize tail is
    # all that remains after PE finishes.
    if n_groups >= 8:
        ntail = max(4, n_groups // 4)
        blocks = [list(range(0, n_groups - ntail)),
                  list(range(n_groups - ntail, n_groups))]
    else:
        blocks = [list(range(n_groups))]
    bmax = max(len(b) for b in blocks)

    nc = bacc.Bacc("TRN2", target_bir_lowering=False, debug=False,
                   enable_asserts=False, num_devices=n_cores)

    PK = nc.dram_tensor("pk", [P, ch * 384], dt.uint8, kind="ExternalInput").ap()
    NTB = nc.dram_tensor("ntb", [P, nodes_per_core], dt.bfloat16,
                         kind="ExternalInput").ap()
    W1P = nc.dram_tensor("w1p", [P, 1024], dt.bfloat16, kind="ExternalInput").ap()
    W2P = nc.dram_tensor("w2p", [P, 512], dt.bfloat16, kind="ExternalInput").ap()
    B1P = nc.dram_tensor("b1p", [P, 4], dt.float32, kind="ExternalInput").ap()
    B2P = nc.dram_tensor("b2p", [P, 1], dt.float32, kind="ExternalInput").ap()
    GAM = nc.dram_tensor("gam", [P, 1], dt.float32, kind="ExternalInput").ap()
    BET = nc.dram_tensor("bet", [P, 1], dt.float32, kind="ExternalInput").ap()
    ONB = nc.dram_tensor("onb", [P, bmax * 128], dt.bfloat16,
                         kind="ExternalInput").ap()
    OUT = nc.dram_tensor("out", [P, nodes_per_core], dt.bfloat16,
                         kind="ExternalOutput").ap()

    with tile.TileContext(nc) as tc:
        with ExitStack() as ctx:
            singles = ctx.enter_context(tc.tile_pool(name="singles", bufs=1))
            ebp = ctx.enter_context(tc.tile_pool(name="ebp", bufs=4))
            xtp = ctx.enter_context(tc.tile_pool(name="xtp", bufs=n_groups + 2))
            xap = ctx.enter_context(tc.tile_pool(name="xap", bufs=4))
            shp = ctx.enter_context(tc.tile_pool(name="shp", bufs=2))
            yp = ctx.enter_context(tc.tile_pool(name="yp", bufs=n_groups + 2))
            y2p = ctx.enter_context(tc.tile_pool(name="y2p", bufs=bmax + 2))
            zp = ctx.enter_context(tc.tile_pool(name="zp", bufs=4))
            stp = ctx.enter_context(tc.tile_pool(name="stp", bufs=1))
            psagg = ctx.enter_context(tc.tile_pool(name="psagg", bufs=2, space="PSUM"))
            psh = ctx.enter_context(tc.tile_pool(name="psh", bufs=3, space="PSUM"))
            psy = ctx.enter_context(tc.tile_pool(name="psy", bufs=1, space="PSUM"))
            psst = ctx.enter_context(tc.tile_pool(name="psst", bufs=1, space="PSUM"))
            drp = ctx.enter_context(tc.tile_pool(name="drp", bufs=2, space="DRAM"))

            def load_const(name, src, shape, dtyp):
                t = singles.tile(shape, dtyp, tag=name)
                nc.sync.dma_start(out=t[:], in_=src)
                return t

            w1 = load_const("w1", W1P, [P, 1024], dt.bfloat16)
            w2 = load_const("w2", W2P, [P, 512], dt.bfloat16)
            b1 = load_const("b1", B1P, [P, 4], dt.float32)
            b2 = load_const("b2", B2P, [P, 1], dt.float32)
            gam = load_const("gam", GAM, [P, 1], dt.float32)
            bet = load_const("bet", BET, [P, 1], dt.float32)
            onb = load_const("onb", ONB, [P, bmax * 128], dt.bfloat16)
            eps = singles.tile([P, 1], dt.float32, tag="eps")
            nc.vector.memset(eps[:], EPS)

            y_tiles = {}
            y2_tiles = {}
            _last_stats = []

            xta_tiles = {}
            xtn_tiles = {}

            def phase1(block, bi, interleave=None):
                bsz = len(block)
                mu_ps = psst.tile([P, GROUP], dt.float32, tag="mups")
                m2_ps = psst.tile([P, GROUP], dt.float32, tag="m2ps")
                for gi, g in enumerate(block):
                    nsl = slice(g * GROUP, (g + 1) * GROUP)
                    xtn = xtp.tile([P, GROUP], dt.bfloat16, tag="xtn")
                    nc.sync.dma_start(out=xtn[:], in_=NTB[:, nsl])
                    xtn_tiles[g] = xtn

                    agg_ps = psagg.tile([P, GROUP], dt.float32, tag="agg")
                    g0 = int(coff[g * 4])
                    gbytes = (int(coff[g * 4 + 4]) - g0) * 384
                    pk = ebp.tile([P, gbytes_max], dt.uint8, tag="pk")
                    nc.sync.dma_start(out=pk[:, :gbytes],
                                      in_=PK[:, g0 * 384:g0 * 384 + gbytes])
                    for t4 in range(4):
                        ti = g * 4 + t4
                        ci = int(cis[ti])
                        toff = (int(coff[ti]) - g0) * 384
                        ebv = pk[:, toff:toff + ci * 256].bitcast(dt.bfloat16)
                        ohv = pk[:, toff + ci * 256:toff + ci * 384].bitcast(
                            dt.float8e4)
                        for c in range(ci):
                            nc.tensor.matmul(
                                out=agg_ps[:, t4 * 128:(t4 + 1) * 128],
                                lhsT=ebv[:, c * 128:(c + 1) * 128],
                                rhs=ohv[:, c * 128:(c + 1) * 128],
                                start=(c == 0), stop=(c == ci - 1))
                    xta = xap.tile([P, GROUP], dt.bfloat16, tag="xta")
                    if g % 2 == 0:
                        nc.scalar.activation(out=xta[:], in_=agg_ps[:], func=AF.Copy)
                    else:
                        nc.vector.tensor_copy(out=xta[:], in_=agg_ps[:])
                    sh_tiles = []
                    for j in range(4):
                        hps = psh.tile([P, GROUP], dt.float32, tag="hps")
                        nc.tensor.matmul(out=hps[:],
                                         lhsT=w1[:, j * 128:(j + 1) * 128],
                                         rhs=xtn[:], start=True, stop=False)
                        nc.tensor.matmul(
                            out=hps[:],
                            lhsT=w1[:, 512 + j * 128:512 + (j + 1) * 128],
                            rhs=xta[:], start=False, stop=True)
                        sh = shp.tile([P, GROUP], dt.bfloat16, tag=f"sh{j}")
                        if act == "silu":
                            nc.scalar.activation(out=sh[:], in_=hps[:],
                                                 func=AF.Silu,
                                                 bias=b1[:, j:j + 1], scale=1.0)
                        else:
                            sg = shp.tile([P, GROUP], dt.float32, tag=f"sg{j}")
                            nc.scalar.activation(out=sg[:], in_=hps[:],
                                                 func=AF.Sigmoid,
                                                 bias=b1[:, j:j + 1], scale=1.0)
                            u = shp.tile([P, GROUP], dt.float32, tag=f"u{j}")
                            nc.vector.tensor_scalar(
                                out=u[:], in0=hps[:], scalar1=b1[:, j:j + 1],
                                scalar2=None, op0=ALU.add)
                            nc.vector.tensor_tensor(out=sh[:], in0=u[:],
                                                    in1=sg[:], op=ALU.mult)
                        sh_tiles.append(sh)

                    yps = psy.tile([P, GROUP], dt.float32, tag="yps")
                    for j in range(4):
                        nc.tensor.matmul(out=yps[:],
                                         lhsT=w2[:, j * 128:(j + 1) * 128],
                                         rhs=sh_tiles[j][:],
                                         start=(j == 0), stop=(j == 3))
                    y = yp.tile([P, GROUP], dt.bfloat16, tag="y")
                    nc.vector.tensor_scalar(out=y[:], in0=yps[:],
                                            scalar1=b2[:, 0:1], scalar2=None,
                                            op0=ALU.add)
                    y_tiles[g] = y
                    y2 = y2p.tile([P, GROUP], dt.bfloat16, tag="y2")
                    nc.vector.tensor_tensor(out=y2[:], in0=y[:], in1=y[:],
                                            op=ALU.mult)
                    y2_tiles[g] = y2
                # block-end stats burst (keeps stats matmuls off the
                # per-group PE critical path)
                for gi, g in enumerate(block):
                    onc_g = onb[:, gi * 128:(gi + 1) * 128]
                    nc.tensor.matmul(out=mu_ps[:], lhsT=onc_g,
                                     rhs=y_tiles[g][:],
                                     start=(gi == 0), stop=(gi == bsz - 1),
                                     skip_group_check=True)
                    nc.tensor.matmul(out=m2_ps[:], lhsT=onc_g,
                                     rhs=y2_tiles.pop(g)[:],
                                     start=(gi == 0), stop=(gi == bsz - 1),
                                     skip_group_check=True)
                _last_stats.append((mu_ps, m2_ps))

            def phase2(block, bi, mu_ps, m2_ps):
                mu_bf = stp.tile([P, GROUP], dt.bfloat16, tag="mubf")
                nc.scalar.activation(out=mu_bf[:], in_=mu_ps[:], func=AF.Copy)
                m2_bf = stp.tile([P, GROUP], dt.bfloat16, tag="m2bf")
                nc.scalar.activation(out=m2_bf[:], in_=m2_ps[:], func=AF.Copy)
                musq = stp.tile([P, GROUP], dt.bfloat16, tag="musq")
                nc.scalar.square(out=musq[:], in_=mu_bf[:])
                var = stp.tile([P, GROUP], dt.bfloat16, tag="var")
                nc.vector.tensor_tensor(out=var[:], in0=m2_bf[:], in1=musq[:],
                                        op=ALU.subtract)
                lnv = stp.tile([P, GROUP], dt.bfloat16, tag="lnv")
                nc.scalar.activation(out=lnv[:], in_=var[:], func=AF.Ln,
                                     bias=eps[:, 0:1], scale=1.0)
                rstd = stp.tile([P, GROUP], dt.bfloat16, tag="rstd")
                nc.scalar.activation(out=rstd[:], in_=lnv[:], func=AF.Exp,
                                     bias=0.0, scale=-0.5)
                bounce = drp.tile([len(block), 1024], dt.bfloat16, tag="bounce")
                nc.gpsimd.dma_start(out=bounce[:, 0:512],
                                    in_=mu_bf[0:len(block), :])
                nc.gpsimd.dma_start(out=bounce[:, 512:1024],
                                    in_=rstd[0:len(block), :])
                return bounce

            def phase3_group(g, gi, bounce):
                    nsl = slice(g * GROUP, (g + 1) * GROUP)
                    mr = zp.tile([P, 1024], dt.bfloat16, tag="mr")
                    bsl = bounce[gi:gi + 1, 0:1024]
                    nc.gpsimd.dma_start(out=mr[:], in_=bass.AP(
                        tensor=bsl.tensor, offset=bsl.offset,
                        ap=[[0, P], bsl.ap[1]]))
                    y = y_tiles.pop(g)
                    xtn = xtn_tiles.pop(g)
                    za = zp.tile([P, GROUP], dt.bfloat16, tag="za")
                    nc.vector.tensor_tensor(out=za[:], in0=y[:],
                                            in1=mr[:, 0:512], op=ALU.subtract)
                    zb = zp.tile([P, GROUP], dt.bfloat16, tag="zb")
                    nc.vector.tensor_tensor(out=zb[:], in0=za[:],
                                            in1=mr[:, 512:1024], op=ALU.mult)
                    zc = zp.tile([P, GROUP], dt.bfloat16, tag="zc")
                    nc.vector.tensor_scalar(out=zc[:], in0=zb[:],
                                            scalar1=gam[:, 0:1],
                                            scalar2=bet[:, 0:1],
                                            op0=ALU.mult, op1=ALU.add)
                    of = zp.tile([P, GROUP], dt.bfloat16, tag="of")
                    nc.vector.tensor_tensor(out=of[:], in0=zc[:], in1=xtn[:],
                                            op=ALU.add)
                    nc.gpsimd.dma_start(out=OUT[:, nsl], in_=of[:])

            # emission: P1(b) P2(b) P3(b). P3 is DVE+DMA-only; with the
            # stats burst at block end, P1(b+1)'s PE work has no DVE
            # dependencies that queue behind P3(b)'s chains.
            for bi, block in enumerate(blocks):
                phase1(block, bi)
                mu_ps, m2_ps = _last_stats.pop()
                bounce = phase2(block, bi, mu_ps, m2_ps)
                for gi, g in enumerate(block):
                    phase3_group(g, gi, bounce)

    nc.compile()
    return nc


# --------------------------------------------------------------------------
# host-side sharding / packing
# --------------------------------------------------------------------------

def _preprocess(inputs, n_cores, nodes_per_core):
    nf = np.ascontiguousarray(np.asarray(inputs["node_features"], np.float32))
    ef = np.ascontiguousarray(np.asarray(inputs["edge_features"], np.float32))
    src = np.asarray(inputs["src_indices"]).astype(np.int64)
    W1 = np.asarray(inputs["W1"], np.float32)
    b1 = np.asarray(inputs["b1"], np.float32)
    W2 = np.asarray(inputs["W2"], np.float32)
    b2 = np.asarray(inputs["b2"], np.float32)
    gam = np.asarray(inputs["ln_gamma"], np.float32)
    bet = np.asarray(inputs["ln_beta"], np.float32)

    n_nodes, d = nf.shape
    n_edges = ef.shape[0]
    tiles_per_core = nodes_per_core // P
    n_groups = nodes_per_core // GROUP
    if n_groups >= 8:
        bmax = n_groups - max(4, n_groups // 4)
    else:
        bmax = n_groups

    order = np.argsort(src, kind="stable")
    snode = src[order]
    core = snode // nodes_per_core
    tile_in_core = (snode % nodes_per_core) // P
    lid = snode % P
    pt = core * tiles_per_core + tile_in_core
    counts = np.bincount(pt, minlength=n_cores * tiles_per_core)
    # per-position chunk counts, shared across cores (SPMD uniformity)
    ccounts = np.ceil(counts.reshape(n_cores, tiles_per_core) / P).astype(int)
    cis = np.maximum(ccounts.max(axis=0), 1)
    coff = np.concatenate([[0], np.cumsum(cis)]).astype(int)
    ch = int(coff[-1])
    cmaxt = int(cis.max())

    starts = np.zeros(n_cores * tiles_per_core, np.int64)
    np.cumsum(counts[:-1], out=starts[1:])
    rank = np.arange(n_edges, dtype=np.int64) - starts[pt]
    chunk = rank // P
    p = rank % P
    cg = coff[tile_in_core] + chunk
    row = core * (P * ch) + p * ch + cg

    ebuf = np.zeros((n_cores * P * ch, d), np.float32)
    ebuf[row] = ef[order]
    EB8 = ebuf.reshape(n_cores, P, ch * d).astype(BF16).view(np.uint8)
    ohbuf = np.zeros((n_cores * P * ch, 128), FP8)
    ohbuf[row, lid] = 1.0
    OH8 = ohbuf.reshape(n_cores, P, ch * 128).view(np.uint8)
    parts = []
    for ti in range(tiles_per_core):
        a, b = int(coff[ti]), int(coff[ti + 1])
        parts.append(EB8[:, :, a * 256:b * 256])
        parts.append(OH8[:, :, a * 128:b * 128])
    PKa = np.ascontiguousarray(np.concatenate(parts, axis=2))

    nfp = np.zeros((n_cores * nodes_per_core, d), np.float32)
    nfp[:n_nodes] = nf
    NTBa = np.ascontiguousarray(
        nfp.reshape(n_cores, nodes_per_core, d).transpose(0, 2, 1)).astype(BF16)

    W1P = np.ascontiguousarray(
        W1.reshape(2, P, 4, P).transpose(1, 0, 2, 3).reshape(P, 1024)).astype(BF16)
    W2P = np.ascontiguousarray(
        W2.reshape(4, P, P).transpose(1, 0, 2).reshape(P, 512)).astype(BF16)
    B1P = np.ascontiguousarray(b1.reshape(4, P).T)
    B2P = np.ascontiguousarray(b2.reshape(P, 1))
    GAMP = np.ascontiguousarray(gam.reshape(P, 1))
    BETP = np.ascontiguousarray(bet.reshape(P, 1))
    ONB = np.zeros((P, bmax * 128), np.float32)
    for g in range(bmax):
        ONB[:, g * 128 + g] = 1.0 / P
    ONB = ONB.astype(BF16)

    in_maps = []
    for k in range(n_cores):
        in_maps.append({
            "pk": PKa[k], "ntb": NTBa[k],
            "w1p": W1P, "w2p": W2P, "b1p": B1P, "b2p": B2P,
            "gam": GAMP, "bet": BETP, "onb": ONB,
        })
    return in_maps, tuple(int(c) for c in cis)


def _assemble(results, n_nodes, n_cores, nodes_per_core):
    outs = np.stack([np.asarray(r["out"]) for r in results])
    full = outs.astype(np.float32).transpose(0, 2, 1).reshape(
        n_cores * nodes_per_core, -1)
    return np.ascontiguousarray(full[:n_nodes])


# --------------------------------------------------------------------------
# public entry point
# --------------------------------------------------------------------------

ACT_MODE = "silu"

_AXON_SO = "/opt/axon/libaxon_pjrt.so"


def _ensure_ntff_hook():
    """Provide antenv.axon_hooks + register the ctypes NTFF profile hook
    (the agent image's antenv lacks axon_hooks, so boot degraded silently)."""
    import sys
    import types
    import ctypes
    import contextlib
    import os

    try:
        from antenv.axon_hooks import get_axon_ntff_profile_hook  # noqa: F401
        return
    except ImportError:
        pass
    import antenv

    m = types.ModuleType("antenv.axon_hooks")
    m._hook = None

    def set_axon_ntff_profile_hook(h):
        m._hook = h

    def get_axon_ntff_profile_hook():
        return m._hook

    m.set_axon_ntff_profile_hook = set_axon_ntff_profile_hook
    m.get_axon_ntff_profile_hook = get_axon_ntff_profile_hook
    sys.modules["antenv.axon_hooks"] = m
    antenv.axon_hooks = m

    if not os.path.exists(_AXON_SO):
        return
    lib = ctypes.CDLL(_AXON_SO)
    if not hasattr(lib, "axon_start_nrt_profile"):
        return
    lib.axon_start_nrt_profile.argtypes = [ctypes.POINTER(ctypes.c_int64),
                                           ctypes.c_size_t]
    lib.axon_start_nrt_profile.restype = ctypes.c_int64
    lib.axon_stop_nrt_profile.argtypes = [ctypes.c_char_p]
    lib.axon_stop_nrt_profile.restype = ctypes.c_int64

    @contextlib.contextmanager
    def _hook(output_dir, device_ids):
        import jax

        jax.devices()
        if device_ids:
            ids = (ctypes.c_int64 * len(device_ids))(*device_ids)
            rc = lib.axon_start_nrt_profile(ids, len(device_ids))
        else:
            rc = lib.axon_start_nrt_profile(None, 0)
        if rc != 0:
            raise RuntimeError(f"axon_start_nrt_profile rc={rc}")
        try:
            yield
        finally:
            n = lib.axon_stop_nrt_profile(str(output_dir).encode())
            if n < 0:
                raise RuntimeError(f"axon_stop_nrt_profile rc={n}")
            if n == 0:
                print("WARNING: NTFF capture wrote no files")

    m._hook = _hook


def _run(inputs, trace=False):
    if trace:
        _ensure_ntff_hook()
    n_nodes = np.asarray(inputs["node_features"]).shape[0]
    in_maps, cis = _preprocess(inputs, N_CORES, NODES_PER_CORE)
    nc = _build(NODES_PER_CORE, cis, N_CORES, ACT_MODE)
    res = bass_utils.run_bass_kernel_spmd(
        nc, in_maps, core_ids=list(range(N_CORES)), trace=trace)
    out = _assemble(res.results, n_nodes, N_CORES, NODES_PER_CORE)
    return out, res


def kernel(**inputs):
    out, _ = _run(inputs, trace=False)
    return out


def kernel_profiled(**inputs):
    out, res = _run(inputs, trace=True)
    return out, res


# revision 36
# speedup vs baseline: 1.2077x; 1.0137x over previous
"""Trainium2 Bass kernel for nn_MeshNodeBlock (GNN message passing block).

reference semantics:
    agg = segment_sum(edge_features, src_indices, N)        # scatter-add
    x   = concat([node_features, agg], -1)
    h   = silu(x @ W1 + b1)
    y   = h @ W2 + b2
    y   = layer_norm(y) * gamma + beta
    out = y + node_features

Strategy (8 NeuronCores, SPMD, one NEFF):
  * Host graph-partitions nodes contiguously across cores (12800 node slots
    per core) and stable-sorts edges by destination node; each core receives
    exactly the edge rows destined for its nodes, grouped by 128-node tile
    and padded to a per-tile-position chunk count C_i (shared across cores
    so the SPMD program is uniform; pad rows are zero).
  * Device works fully in transposed space (features on partitions, nodes on
    free dim). Per 128-node tile the scatter-add is C_i PE matmuls
    aggT += edge_chunk.T @ onehot. One-hot blocks for a whole tile are built
    in one 2x-mode vector is_equal against a tiled-iota constant, with the
    local ids pre-expanded by a gpsimd broadcast copy.
  * MLP consumes aggT/nodeT directly: layer 1 -> hT_j slices, silu(+b1) on
    the scalar engine, layer 2 -> yT.
  * LayerNorm stats via matmuls whose lhsT is a block-diagonal 1/128 column
    (ONCB): group g's mean/mean-of-squares land on PSUM row g of a shared
    bank, accumulated over a block of groups. Stats post-processing
    (var, rstd=exp(-0.5*ln(var+eps))) runs once per block at full width,
    then rows bounce through a DRAM tile and DMA-broadcast back across
    partitions. Processing is phase-blocked to minimize ACT table switches.
  * Output written transposed in bf16; host transposes/casts back.
"""

import functools
from contextlib import ExitStack

import numpy as np
import ml_dtypes

import concourse.bass as bass
import concourse.tile as tile
from concourse import bacc, mybir
from concourse import bass_utils

BF16 = ml_dtypes.bfloat16
FP8 = ml_dtypes.float8_e4m3

N_NODES = 100000
D = 128
N_CORES = 8
P = 128
GROUP = 512              # nodes per group = 4 tiles
NODES_PER_CORE = 12800   # 25 groups
C_MAX = 8                # fallback chunk budget per tile (exact counts used)
NBLK = 2                 # phase blocks
INTERLEAVE_P3 = True    # interleave prev block's normalize into next phase1
EPS = 1e-5

AF = mybir.ActivationFunctionType
ALU = mybir.AluOpType
dt = mybir.dt


# --------------------------------------------------------------------------
# device kernel builder
# --------------------------------------------------------------------------

@functools.lru_cache(maxsize=4)
def _build(nodes_per_core: int, cis: tuple, n_cores: int, act: str = "silu"):
    assert nodes_per_core % GROUP == 0
    n_groups = nodes_per_core // GROUP
    tiles_per_core = nodes_per_core // P
    assert len(cis) == tiles_per_core
    coff = np.concatenate([[0], np.cumsum(cis)]).astype(int)
    ch = int(coff[-1])                   # total chunks per core
    cmaxt = int(max(cis))
    gbytes_max = max(
        (int(coff[gg * 4 + 4]) - int(coff[gg * 4])) * 384
        for gg in range(nodes_per_core // GROUP))

    # phase blocks of groups (ACT table switches cost ~2.7us per set swap).
    # Asymmetric: big first block, small last block whose normalize tail is
    # all that remains after PE finishes.
    if n_groups >= 8:
        ntail = 4
        blocks = [list(range(0, n_groups - ntail)),
                  list(range(n_groups - ntail, n_groups))]
    else:
        blocks = [list(range(n_groups))]
    bmax = max(len(b) for b in blocks)

    nc = bacc.Bacc("TRN2", target_bir_lowering=False, debug=False,
                   enable_asserts=False, num_devices=n_cores)

    PK = nc.dram_tensor("pk", [P, ch * 384], dt.uint8, kind="ExternalInput").ap()
    NTB = nc.dram_tensor("ntb", [P, nodes_per_core], dt.bfloat16,
                         kind="ExternalInput").ap()
    W1P = nc.dram_tensor("w1p", [P, 1024], dt.bfloat16, kind="ExternalInput").ap()
    W2P = nc.dram_tensor("w2p", [P, 512], dt.bfloat16, kind="ExternalInput").ap()
    B1P = nc.dram_tensor("b1p", [P, 4], dt.float32, kind="ExternalInput").ap()
    B2P = nc.dram_tensor("b2p", [P, 1], dt.float32, kind="ExternalInput").ap()
    GAM = nc.dram_tensor("gam", [P, 1], dt.float32, kind="ExternalInput").ap()
    BET = nc.dram_tensor("bet", [P, 1], dt.float32, kind="ExternalInput").ap()
    ONB = nc.dram_tensor("onb", [P, bmax * 128], dt.bfloat16,
                         kind="ExternalInput").ap()
    OUT = nc.dram_tensor("out", [P, nodes_per_core], dt.bfloat16,
                         kind="ExternalOutput").ap()

    with tile.TileContext(nc) as tc:
        with ExitStack() as ctx:
            singles = ctx.enter_context(tc.tile_pool(name="singles", bufs=1))
            ebp = ctx.enter_context(tc.tile_pool(name="ebp", bufs=4))
            xtp = ctx.enter_context(tc.tile_pool(name="xtp", bufs=n_groups + 2))
            xap = ctx.enter_context(tc.tile_pool(name="xap", bufs=4))
            shp = ctx.enter_context(tc.tile_pool(name="shp", bufs=2))
            yp = ctx.enter_context(tc.tile_pool(name="yp", bufs=n_groups + 2))
            y2p = ctx.enter_context(tc.tile_pool(name="y2p", bufs=bmax + 2))
            zp = ctx.enter_context(tc.tile_pool(name="zp", bufs=6))
            stp = ctx.enter_context(tc.tile_pool(name="stp", bufs=1))
            psagg = ctx.enter_context(tc.tile_pool(name="psagg", bufs=2, space="PSUM"))
            psh = ctx.enter_context(tc.tile_pool(name="psh", bufs=3, space="PSUM"))
            psy = ctx.enter_context(tc.tile_pool(name="psy", bufs=1, space="PSUM"))
            psst = ctx.enter_context(tc.tile_pool(name="psst", bufs=1, space="PSUM"))
            drp = ctx.enter_context(tc.tile_pool(name="drp", bufs=2, space="DRAM"))

            def load_const(name, src, shape, dtyp):
                t = singles.tile(shape, dtyp, tag=name)
                nc.sync.dma_start(out=t[:], in_=src)
                return t

            w1 = load_const("w1", W1P, [P, 1024], dt.bfloat16)
            w2 = load_const("w2", W2P, [P, 512], dt.bfloat16)
            b1 = load_const("b1", B1P, [P, 4], dt.float32)
            b2 = load_const("b2", B2P, [P, 1], dt.float32)
            gam = load_const("gam", GAM, [P, 1], dt.float32)
            bet = load_const("bet", BET, [P, 1], dt.float32)
            onb = load_const("onb", ONB, [P, bmax * 128], dt.bfloat16)
            eps = singles.tile([P, 1], dt.float32, tag="eps")
            nc.vector.memset(eps[:], EPS)

            y_tiles = {}
            y2_tiles = {}
            _last_stats = []

            xta_tiles = {}
            xtn_tiles = {}

            def phase1(block, bi, interleave=None):
                bsz = len(block)
                mu_ps = psst.tile([P, GROUP], dt.float32, tag="mups")
                m2_ps = psst.tile([P, GROUP], dt.float32, tag="m2ps")
                for gi, g in enumerate(block):
                    nsl = slice(g * GROUP, (g + 1) * GROUP)
                    xtn = xtp.tile([P, GROUP], dt.bfloat16, tag="xtn")
                    nc.sync.dma_start(out=xtn[:], in_=NTB[:, nsl])
                    xtn_tiles[g] = xtn

                    agg_ps = psagg.tile([P, GROUP], dt.float32, tag="agg")
                    g0 = int(coff[g * 4])
                    gbytes = (int(coff[g * 4 + 4]) - g0) * 384
                    pk = ebp.tile([P, gbytes_max], dt.uint8, tag="pk")
                    nc.sync.dma_start(out=pk[:, :gbytes],
                                      in_=PK[:, g0 * 384:g0 * 384 + gbytes])
                    for t4 in range(4):
                        ti = g * 4 + t4
                        ci = int(cis[ti])
                        toff = (int(coff[ti]) - g0) * 384
                        ebv = pk[:, toff:toff + ci * 256].bitcast(dt.bfloat16)
                        ohv = pk[:, toff + ci * 256:toff + ci * 384].bitcast(
                            dt.float8e4)
                        for c in range(ci):
                            nc.tensor.matmul(
                                out=agg_ps[:, t4 * 128:(t4 + 1) * 128],
                                lhsT=ebv[:, c * 128:(c + 1) * 128],
                                rhs=ohv[:, c * 128:(c + 1) * 128],
                                start=(c == 0), stop=(c == ci - 1))
                    xta = xap.tile([P, GROUP], dt.bfloat16, tag="xta")
                    if g % 2 == 0:
                        nc.scalar.activation(out=xta[:], in_=agg_ps[:], func=AF.Copy)
                    else:
                        nc.vector.tensor_copy(out=xta[:], in_=agg_ps[:])
                    sh_tiles = []
                    for j in range(4):
                        hps = psh.tile([P, GROUP], dt.float32, tag="hps")
                        nc.tensor.matmul(out=hps[:],
                                         lhsT=w1[:, j * 128:(j + 1) * 128],
                                         rhs=xtn[:], start=True, stop=False)
                        nc.tensor.matmul(
                            out=hps[:],
                            lhsT=w1[:, 512 + j * 128:512 + (j + 1) * 128],
                            rhs=xta[:], start=False, stop=True)
                        sh = shp.tile([P, GROUP], dt.bfloat16, tag=f"sh{j}")
                        if act == "silu":
                            nc.scalar.activation(out=sh[:], in_=hps[:],
                                                 func=AF.Silu,
                                                 bias=b1[:, j:j + 1], scale=1.0)
                        else:
                            sg = shp.tile([P, GROUP], dt.float32, tag=f"sg{j}")
                            nc.scalar.activation(out=sg[:], in_=hps[:],
                                                 func=AF.Sigmoid,
                                                 bias=b1[:, j:j + 1], scale=1.0)
                            u = shp.tile([P, GROUP], dt.float32, tag=f"u{j}")
                            nc.vector.tensor_scalar(
                                out=u[:], in0=hps[:], scalar1=b1[:, j:j + 1],
                                scalar2=None, op0=ALU.add)
                            nc.vector.tensor_tensor(out=sh[:], in0=u[:],
                                                    in1=sg[:], op=ALU.mult)
                        sh_tiles.append(sh)

                    yps = psy.tile([P, GROUP], dt.float32, tag="yps")
                    for j in range(4):
                        nc.tensor.matmul(out=yps[:],
                                         lhsT=w2[:, j * 128:(j + 1) * 128],
                                         rhs=sh_tiles[j][:],
                                         start=(j == 0), stop=(j == 3))
                    y = yp.tile([P, GROUP], dt.bfloat16, tag="y")
                    nc.vector.tensor_scalar(out=y[:], in0=yps[:],
                                            scalar1=b2[:, 0:1], scalar2=None,
                                            op0=ALU.add)
                    y_tiles[g] = y
                    y2 = y2p.tile([P, GROUP], dt.bfloat16, tag="y2")
                    nc.vector.tensor_tensor(out=y2[:], in0=y[:], in1=y[:],
                                            op=ALU.mult)
                    y2_tiles[g] = y2
                # block-end stats burst (keeps stats matmuls off the
                # per-group PE critical path)
                for gi, g in enumerate(block):
                    onc_g = onb[:, gi * 128:(gi + 1) * 128]
                    nc.tensor.matmul(out=mu_ps[:], lhsT=onc_g,
                                     rhs=y_tiles[g][:],
                                     start=(gi == 0), stop=(gi == bsz - 1),
                                     skip_group_check=True)
                    nc.tensor.matmul(out=m2_ps[:], lhsT=onc_g,
                                     rhs=y2_tiles.pop(g)[:],
                                     start=(gi == 0), stop=(gi == bsz - 1),
                                     skip_group_check=True)
                _last_stats.append((mu_ps, m2_ps))

            def phase2(block, bi, mu_ps, m2_ps):
                mu_bf = stp.tile([P, GROUP], dt.bfloat16, tag="mubf")
                nc.scalar.activation(out=mu_bf[:], in_=mu_ps[:], func=AF.Copy)
                m2_bf = stp.tile([P, GROUP], dt.bfloat16, tag="m2bf")
                nc.scalar.activation(out=m2_bf[:], in_=m2_ps[:], func=AF.Copy)
                musq = stp.tile([P, GROUP], dt.bfloat16, tag="musq")
                nc.scalar.square(out=musq[:], in_=mu_bf[:])
                var = stp.tile([P, GROUP], dt.bfloat16, tag="var")
                nc.vector.tensor_tensor(out=var[:], in0=m2_bf[:], in1=musq[:],
                                        op=ALU.subtract)
                lnv = stp.tile([P, GROUP], dt.bfloat16, tag="lnv")
                nc.scalar.activation(out=lnv[:], in_=var[:], func=AF.Ln,
                                     bias=eps[:, 0:1], scale=1.0)
                rstd = stp.tile([P, GROUP], dt.bfloat16, tag="rstd")
                nc.scalar.activation(out=rstd[:], in_=lnv[:], func=AF.Exp,
                                     bias=0.0, scale=-0.5)
                bounce = drp.tile([len(block), 1024], dt.bfloat16, tag="bounce")
                nc.gpsimd.dma_start(out=bounce[:, 0:512],
                                    in_=mu_bf[0:len(block), :])
                nc.gpsimd.dma_start(out=bounce[:, 512:1024],
                                    in_=rstd[0:len(block), :])
                return bounce

            def phase3_group(g, gi, bounce):
                    nsl = slice(g * GROUP, (g + 1) * GROUP)
                    mr = zp.tile([P, 1024], dt.bfloat16, tag="mr")
                    bsl = bounce[gi:gi + 1, 0:1024]
                    nc.gpsimd.dma_start(out=mr[:], in_=bass.AP(
                        tensor=bsl.tensor, offset=bsl.offset,
                        ap=[[0, P], bsl.ap[1]]))
                    y = y_tiles.pop(g)
                    xtn = xtn_tiles.pop(g)
                    za = zp.tile([P, GROUP], dt.bfloat16, tag="za")
                    nc.vector.tensor_tensor(out=za[:], in0=y[:],
                                            in1=mr[:, 0:512], op=ALU.subtract)
                    zb = zp.tile([P, GROUP], dt.bfloat16, tag="zb")
                    nc.vector.tensor_tensor(out=zb[:], in0=za[:],
                                            in1=mr[:, 512:1024], op=ALU.mult)
                    zc = zp.tile([P, GROUP], dt.bfloat16, tag="zc")
                    nc.vector.tensor_scalar(out=zc[:], in0=zb[:],
                                            scalar1=gam[:, 0:1],
                                            scalar2=bet[:, 0:1],
                                            op0=ALU.mult, op1=ALU.add)
                    of = zp.tile([P, GROUP], dt.bfloat16, tag="of")
                    nc.vector.tensor_tensor(out=of[:], in0=zc[:], in1=xtn[:],
                                            op=ALU.add)
                    nc.gpsimd.dma_start(out=OUT[:, nsl], in_=of[:])

            # emission: P1(b) P2(b) P3(b). P3 is DVE+DMA-only; with the
            # stats burst at block end, P1(b+1)'s PE work has no DVE
            # dependencies that queue behind P3(b)'s chains.
            for bi, block in enumerate(blocks):
                phase1(block, bi)
                mu_ps, m2_ps = _last_stats.pop()
                bounce = phase2(block, bi, mu_ps, m2_ps)
                for gi, g in enumerate(block):
                    phase3_group(g, gi, bounce)

    nc.compile()
    return nc


# --------------------------------------------------------------------------
# host-side sharding / packing
# --------------------------------------------------------------------------

def _preprocess(inputs, n_cores, nodes_per_core):
    nf = np.ascontiguousarray(np.asarray(inputs["node_features"], np.float32))
    ef = np.ascontiguousarray(np.asarray(inputs["edge_features"], np.float32))
    src = np.asarray(inputs["src_indices"]).astype(np.int64)
    W1 = np.asarray(inputs["W1"], np.float32)
    b1 = np.asarray(inputs["b1"], np.float32)
    W2 = np.asarray(inputs["W2"], np.float32)
    b2 = np.asarray(inputs["b2"], np.float32)
    gam = np.asarray(inputs["ln_gamma"], np.float32)
    bet = np.asarray(inputs["ln_beta"], np.float32)

    n_nodes, d = nf.shape
    n_edges = ef.shape[0]
    tiles_per_core = nodes_per_core // P
    n_groups = nodes_per_core // GROUP
    if n_groups >= 8:
        bmax = n_groups - 4
    else:
        bmax = n_groups

    order = np.argsort(src, kind="stable")
    snode = src[order]
    core = snode // nodes_per_core
    tile_in_core = (snode % nodes_per_core) // P
    lid = snode % P
    pt = core * tiles_per_core + tile_in_core
    counts = np.bincount(pt, minlength=n_cores * tiles_per_core)
    # per-position chunk counts, shared across cores (SPMD uniformity)
    ccounts = np.ceil(counts.reshape(n_cores, tiles_per_core) / P).astype(int)
    cis = np.maximum(ccounts.max(axis=0), 1)
    coff = np.concatenate([[0], np.cumsum(cis)]).astype(int)
    ch = int(coff[-1])
    cmaxt = int(cis.max())

    starts = np.zeros(n_cores * tiles_per_core, np.int64)
    np.cumsum(counts[:-1], out=starts[1:])
    rank = np.arange(n_edges, dtype=np.int64) - starts[pt]
    chunk = rank // P
    p = rank % P
    cg = coff[tile_in_core] + chunk
    row = core * (P * ch) + p * ch + cg

    ebuf = np.zeros((n_cores * P * ch, d), np.float32)
    ebuf[row] = ef[order]
    EB8 = ebuf.reshape(n_cores, P, ch * d).astype(BF16).view(np.uint8)
    ohbuf = np.zeros((n_cores * P * ch, 128), FP8)
    ohbuf[row, lid] = 1.0
    OH8 = ohbuf.reshape(n_cores, P, ch * 128).view(np.uint8)
    parts = []
    for ti in range(tiles_per_core):
        a, b = int(coff[ti]), int(coff[ti + 1])
        parts.append(EB8[:, :, a * 256:b * 256])
        parts.append(OH8[:, :, a * 128:b * 128])
    PKa = np.ascontiguousarray(np.concatenate(parts, axis=2))

    nfp = np.zeros((n_cores * nodes_per_core, d), np.float32)
    nfp[:n_nodes] = nf
    NTBa = np.ascontiguousarray(
        nfp.reshape(n_cores, nodes_per_core, d).transpose(0, 2, 1)).astype(BF16)

    W1P = np.ascontiguousarray(
        W1.reshape(2, P, 4, P).transpose(1, 0, 2, 3).reshape(P, 1024)).astype(BF16)
    W2P = np.ascontiguousarray(
        W2.reshape(4, P, P).transpose(1, 0, 2).reshape(P, 512)).astype(BF16)
    B1P = np.ascontiguousarray(b1.reshape(4, P).T)
    B2P = np.ascontiguousarray(b2.reshape(P, 1))
    GAMP = np.ascontiguousarray(gam.reshape(P, 1))
    BETP = np.ascontiguousarray(bet.reshape(P, 1))
    ONB = np.zeros((P, bmax * 128), np.float32)
    for g in range(bmax):
        ONB[:, g * 128 + g] = 1.0 / P
    ONB = ONB.astype(BF16)

    in_maps = []
    for k in range(n_cores):
        in_maps.append({
            "pk": PKa[k], "ntb": NTBa[k],
            "w1p": W1P, "w2p": W2P, "b1p": B1P, "b2p": B2P,
            "gam": GAMP, "bet": BETP, "onb": ONB,
        })
    return in_maps, tuple(int(c) for c in cis)


def _assemble(results, n_nodes, n_cores, nodes_per_core):
    outs = np.stack([np.asarray(r["out"]) for r in results])
    full = outs.astype(np.float32).transpose(0, 2, 1).reshape(
        n_cores * nodes_per_core, -1)
    return np.ascontiguousarray(full[:n_nodes])


# --------------------------------------------------------------------------
# public entry point
# --------------------------------------------------------------------------

ACT_MODE = "silu"

_AXON_SO = "/opt/axon/libaxon_pjrt.so"


def _ensure_ntff_hook():
    """Provide antenv.axon_hooks + register the ctypes NTFF profile hook
    (the agent image's antenv lacks axon_hooks, so boot degraded silently)."""
    import sys
    import types
    import ctypes
    import contextlib
    import os

    try:
        from antenv.axon_hooks import get_axon_ntff_profile_hook  # noqa: F401
        return
    except ImportError:
        pass
    import antenv

    m = types.ModuleType("antenv.axon_hooks")
    m._hook = None

    def set_axon_ntff_profile_hook(h):
        m._hook = h

    def get_axon_ntff_profile_hook():
        return m._hook

    m.set_axon_ntff_profile_hook = set_axon_ntff_profile_hook
    m.get_axon_ntff_profile_hook = get_axon_ntff_profile_hook
    sys.modules["antenv.axon_hooks"] = m
    antenv.axon_hooks = m

    if not os.path.exists(_AXON_SO):
        return
    lib = ctypes.CDLL(_AXON_SO)
    if not hasattr(lib, "axon_start_nrt_profile"):
        return
    lib.axon_start_nrt_profile.argtypes = [ctypes.POINTER(ctypes.c_int64),
                                           ctypes.c_size_t]
    lib.axon_start_nrt_profile.restype = ctypes.c_int64
    lib.axon_stop_nrt_profile.argtypes = [ctypes.c_char_p]
    lib.axon_stop_nrt_profile.restype = ctypes.c_int64

    @contextlib.contextmanager
    def _hook(output_dir, device_ids):
        import jax

        jax.devices()
        if device_ids:
            ids = (ctypes.c_int64 * len(device_ids))(*device_ids)
            rc = lib.axon_start_nrt_profile(ids, len(device_ids))
        else:
            rc = lib.axon_start_nrt_profile(None, 0)
        if rc != 0:
            raise RuntimeError(f"axon_start_nrt_profile rc={rc}")
        try:
            yield
        finally:
            n = lib.axon_stop_nrt_profile(str(output_dir).encode())
            if n < 0:
                raise RuntimeError(f"axon_stop_nrt_profile rc={n}")
            if n == 0:
                print("WARNING: NTFF capture wrote no files")

    m._hook = _hook


def _run(inputs, trace=False):
    if trace:
        _ensure_ntff_hook()
    n_nodes = np.asarray(inputs["node_features"]).shape[0]
    in_maps, cis = _preprocess(inputs, N_CORES, NODES_PER_CORE)
    nc = _build(NODES_PER_CORE, cis, N_CORES, ACT_MODE)
    res = bass_utils.run_bass_kernel_spmd(
        nc, in_maps, core_ids=list(range(N_CORES)), trace=trace)
    out = _assemble(res.results, n_nodes, N_CORES, NODES_PER_CORE)
    return out, res


def kernel(**inputs):
    out, _ = _run(inputs, trace=False)
    return out


def kernel_profiled(**inputs):
    out, res = _run(inputs, trace=True)
    return out, res
